# revision 1
# baseline (speedup 1.0000x reference)
"""Cross-attention (GQA + RoPE) Trainium2 Bass kernel.

Sharding: 8 cores = 4 batches x 2 head-groups.
  core i -> batch b = i // 2, head-group g = i % 2
  Each core computes 8 query heads / 2 kv heads of one batch and a
  row-parallel partial of the output projection; the host sums the two
  partials per batch.

Per-core layout (all "T" tensors have head_dim / feature on partitions):
  qT   [1024, TQ]   query^T               (host-transposed)
  kvT  [1024, TKV]  key_value^T           (host-transposed)
  wq   [1024, 512]  w_q columns of this head group, head-PERMUTED so that
                    pair-tile j holds local heads (j, j+4) -> rows (0-63, 64-127).
                    This makes the Q row base (64*(h//4)) equal the K row base
                    for every head (required: matmul lhsT/rhs partition bases
                    must match the PE row placement).
  wk   [1024, 128]  w_k columns (2 kv heads)
  wv   [1024, 128]  w_v columns
  wout [512, 1024]  w_out rows, same head permutation as wq columns
  cosF [128, TKV]   rope cos stacked [c;c;c;c]   (32 rows repeated)
  sinF [128, TKV]   rope sin stacked [-s;s;-s;s]
  maskb [128, NCH]  additive kv-mask bias per 128-chunk (0 / -30000)

Algorithm per core:
  K^T = rope(wk^T @ kvT)      resident [128, TKV]   (2 kv heads stacked)
  V   = (kvT chunks)^T @ wv   resident [128, 65*NCH] per kv head, with an
                              appended ones-column per chunk (softmax denom)
  per tq block T2, per head:
     scores^T chunk [tkv 128, tq T2] = K_c^T.T @ Q^T   (PSUM)
     e = exp(0.125*scores^T + mask_bias)               (ACT, bias per partition)
     psum_o [65, T2] += V_c_aug.T @ e                  (row 64 = sum of exp)
     attnT = psum_o[0:64] * broadcast(1/psum_o[64])    (DVE + gpsimd bcast)
  out[tq, :] partial = attnT.T @ wout                  (PSUM -> DMA)
"""

import os
from contextlib import ExitStack

import numpy as np

import concourse.bass as bass
import concourse.bacc as bacc
import concourse.mybir as mybir
import concourse.tile as tile
from concourse.bass_utils import run_bass_kernel_spmd

F32 = mybir.dt.float32
R32 = mybir.dt.float32r

D_MODEL = 1024
N_HEADS = 16
NUM_KV_HEADS = 4
D_K = 64
ROPE_BASE = 10000.0
B = 4
TQ = 2048
TKV = 2048
N_CORES = 8

NEG_BIAS = -30000.0


def build_bass(tq=TQ, tkv=TKV, t2=1024, use_f32r=True):
    """Build the single-core SPMD program (same program on all 8 cores)."""
    nc = bacc.Bacc("TRN2", target_bir_lowering=False, debug=False)
    P = 128
    NKT = tkv // 512          # kv projection tiles
    NCH = tkv // 128          # attention kv chunks
    NT2 = tq // t2            # tq blocks
    NHALF = t2 // 512         # 512-wide matmul slices per tq block
    NPAIR = 4                 # head-pair tiles per core
    DT = R32 if use_f32r else F32

    qT = nc.dram_tensor("qT", [D_MODEL, tq], DT, kind="ExternalInput").ap()
    kvT = nc.dram_tensor("kvT", [D_MODEL, tkv], DT, kind="ExternalInput").ap()
    wq = nc.dram_tensor("wq", [D_MODEL, 512], DT, kind="ExternalInput").ap()
    wk = nc.dram_tensor("wk", [D_MODEL, 128], DT, kind="ExternalInput").ap()
    wv = nc.dram_tensor("wv", [D_MODEL, 128], DT, kind="ExternalInput").ap()
    wout = nc.dram_tensor("wout", [512, D_MODEL], DT, kind="ExternalInput").ap()
    cosF = nc.dram_tensor("cosF", [P, tkv], F32, kind="ExternalInput").ap()
    sinF = nc.dram_tensor("sinF", [P, tkv], F32, kind="ExternalInput").ap()
    maskb = nc.dram_tensor("maskb", [P, NCH], F32, kind="ExternalInput").ap()
    onesc = nc.dram_tensor("onesc", [P, 64], DT, kind="ExternalInput").ap()
    out = nc.dram_tensor("out", [tq, D_MODEL], F32, kind="ExternalOutput").ap()

    with tile.TileContext(nc) as tc, ExitStack() as ctx:
        const = ctx.enter_context(tc.tile_pool(name="const", bufs=1))
        blkp = ctx.enter_context(tc.tile_pool(name="blkp", bufs=2))
        qpool = ctx.enter_context(tc.tile_pool(name="qpool", bufs=1))
        apool = ctx.enter_context(tc.tile_pool(name="apool", bufs=1))
        workp = ctx.enter_context(tc.tile_pool(name="workp", bufs=3))
        ropep = ctx.enter_context(tc.tile_pool(name="ropep", bufs=2))
        outp = ctx.enter_context(tc.tile_pool(name="outp", bufs=2))
        pp_big = ctx.enter_context(tc.tile_pool(name="pp_big", bufs=2, space="PSUM"))
        pp_acc = ctx.enter_context(tc.tile_pool(name="pp_acc", bufs=2, space="PSUM"))

        def MM(out_ap, lhsT, rhs, start, stop, chain=None):
            inst = nc.tensor.matmul(out_ap, lhsT, rhs, start=start, stop=stop)
            if chain is not None:
                tc.chain_iter_dep(chain, inst.ins)
            return inst

        def chain_dve(inst):
            tc.chain_iter_dep("dve_norm", inst.ins)
            return inst

        # ---- constants / weights -------------------------------------------------
        wq_sb = const.tile([P, 8, 512], DT)
        nc.gpsimd.dma_start(out=wq_sb, in_=wq.rearrange("(c p) f -> p c f", p=P))
        wk_sb = const.tile([P, 8, 128], DT)
        nc.gpsimd.dma_start(out=wk_sb, in_=wk.rearrange("(c p) f -> p c f", p=P))
        wv_sb = const.tile([P, 8, 128], DT)
        nc.gpsimd.dma_start(out=wv_sb, in_=wv.rearrange("(c p) f -> p c f", p=P))
        wout_sb = const.tile([P, 4, D_MODEL], DT)
        nc.gpsimd.dma_start(out=wout_sb, in_=wout.rearrange("(c p) f -> p c f", p=P))
        cos_sb = const.tile([P, tkv], F32)
        nc.gpsimd.dma_start(out=cos_sb, in_=cosF)
        sin_sb = const.tile([P, tkv], F32)
        nc.gpsimd.dma_start(out=sin_sb, in_=sinF)
        mask_sb = const.tile([P, NCH], F32)
        nc.gpsimd.dma_start(out=mask_sb, in_=maskb)

        Kt = const.tile([P, tkv], DT)
        Vt = [const.tile([P, NCH * 65], DT, name=f"Vt{i}") for i in range(2)]
        for i in range(2):
            nc.gpsimd.dma_start(
                out=Vt[i].rearrange("p (c k) -> p c k", k=65)[:, :, 64],
                in_=onesc[:, :NCH],
            )
        ones_sb = const.tile([1, 64], DT)
        nc.gpsimd.dma_start(out=ones_sb, in_=onesc[0:1, :])

        def rope_apply(dest, ps, col0, width):
            """dest[128, width] (SBUF) = rope(ps[128, width] PSUM), positions
            col0..col0+width. Rows are two stacked heads, each [x1(32); x2(32)]."""
            cs = cos_sb[:, col0 : col0 + width]
            t_cos = ropep.tile([P, t2], F32, tag="rope", name="t_cos")
            t_u = ropep.tile([P, t2], F32, tag="rope", name="t_u")
            tc_ = t_cos[:, :width]
            tu_ = t_u[:, :width]
            nc.vector.tensor_mul(tc_, ps, cs)
            for b0 in (0, 64):
                # sinF rows [b0:b0+32] = -sin, [b0+32:b0+64] = +sin
                nc.vector.tensor_mul(
                    tu_[b0 : b0 + 32, :],
                    ps[b0 + 32 : b0 + 64, :],
                    sin_sb[b0 : b0 + 32, col0 : col0 + width],
                )
                nc.vector.tensor_mul(
                    tu_[b0 + 32 : b0 + 64, :],
                    ps[b0 : b0 + 32, :],
                    sin_sb[b0 + 32 : b0 + 64, col0 : col0 + width],
                )
            nc.vector.tensor_add(dest, tc_, tu_)

        # ---- phase KV: K/V projections ------------------------------------------
        for kt in range(NKT):
            kv_blk = blkp.tile([P, 8, 512], DT, tag="blk", name="kv_blk")
            nc.gpsimd.dma_start(
                out=kv_blk,
                in_=kvT.rearrange("(c p) t -> p c t", p=P)[
                    :, :, kt * 512 : (kt + 1) * 512
                ],
            )
            ps_k = pp_big.tile([P, 512], F32, tag="big", name="ps_k")
            for d in range(8):
                MM(ps_k, wk_sb[:, d, :], kv_blk[:, d, :], d == 0, d == 7)
            rope_apply(Kt[:, kt * 512 : (kt + 1) * 512], ps_k, kt * 512, 512)
            for s in range(4):
                ps_v = pp_big.tile([P, 512], F32, tag="big", name="ps_v")
                pv = ps_v[:, 0:128]
                for d in range(8):
                    MM(
                        pv,
                        kv_blk[:, d, s * 128 : (s + 1) * 128],
                        wv_sb[:, d, :],
                        d == 0,
                        d == 7,
                    )
                c = kt * 4 + s
                nc.vector.tensor_copy(
                    out=Vt[0][:, c * 65 : c * 65 + 64], in_=pv[:, 0:64]
                )
                nc.vector.tensor_copy(
                    out=Vt[1][:, c * 65 : c * 65 + 64], in_=pv[:, 64:128]
                )

        # ---- per tq block: Q proj -> attention -> output projection -------------
        for it2 in range(NT2):
            q_blks = []
            for half in range(NHALF):
                qb = blkp.tile([P, 8, 512], DT, tag="blk", name="q_blk")
                c0 = it2 * t2 + half * 512
                nc.gpsimd.dma_start(
                    out=qb,
                    in_=qT.rearrange("(c p) t -> p c t", p=P)[:, :, c0 : c0 + 512],
                )
                q_blks.append(qb)

            Qt = []
            for j in range(NPAIR):
                ps_q = pp_big.tile([P, t2], F32, tag="big", name="ps_q")
                for half in range(NHALF):
                    for d in range(8):
                        MM(
                            ps_q[:, half * 512 : (half + 1) * 512],
                            wq_sb[:, d, j * 128 : (j + 1) * 128],
                            q_blks[half][:, d, :],
                            d == 0,
                            d == 7,
                        )
                qt = qpool.tile([P, t2], DT, tag=f"Q{j}", name=f"Qt{j}")
                rope_apply(qt, ps_q, it2 * t2, t2)
                Qt.append(qt)

            attnT = [
                apool.tile([P, t2], DT, tag=f"A{j}", name=f"attnT{j}")
                for j in range(NPAIR)
            ]

            # normalization of head h is EMITTED after head h+1's attention
            # matmuls: the broadcast matmul would otherwise head-of-line block
            # the in-order PE queue on the (slow, [1,t2]) DVE reciprocal.
            pending = []

            def flush_norm():
                if not pending:
                    return
                U, inv, j_, base_ = pending.pop(0)
                for half in range(NHALF):
                    hs = slice(half * 512, (half + 1) * 512)
                    ps_b = pp_big.tile([64, 512], F32, tag="big", name="ps_b")
                    MM(ps_b, ones_sb, inv[:, hs], True, True, chain="pe_attn")
                    chain_dve(
                        nc.vector.tensor_mul(
                            attnT[j_][base_ : base_ + 64, hs], U[0:64, hs], ps_b
                        )
                    )

            # two heads (j, j+4) interleave: while one head's exp is on the
            # Scalar engine, the PE runs the other head's matmuls back-to-back
            # (keeps the PE activity window busy -> HAM stays at K=8/8).
            for j in range(NPAIR):
                heads = [(j, 0, 0), (j + 4, 1, 64)]  # (head, kvh, base)
                ps_os = [
                    pp_acc.tile([65, t2], F32, tag="acc", name=f"ps_o{ab}")
                    for ab in range(2)
                ]
                def emit_pv(c_, exs_):
                    for ab, (_h, kvh, _base) in enumerate(heads):
                        for half in range(NHALF):
                            MM(
                                ps_os[ab][:, half * 512 : (half + 1) * 512],
                                Vt[kvh][:, c_ * 65 : c_ * 65 + 65],
                                exs_[ab][:, half * 512 : (half + 1) * 512],
                                c_ == 0,
                                c_ == NCH - 1,
                                chain="pe_attn",
                            )

                # PV lags the scores by one chunk so no PE instruction ever
                # reaches the queue head with an unresolved wait (embedded
                # stalls keep the HAM activity window cold).
                prev = None
                for c in range(NCH):
                    exs = []
                    for ab, (_h, kvh, base) in enumerate(heads):
                        ps_s = pp_big.tile([P, t2], F32, tag="big", name="ps_s")
                        for half in range(NHALF):
                            MM(
                                ps_s[:, half * 512 : (half + 1) * 512],
                                Kt[base : base + 64, c * 128 : (c + 1) * 128],
                                Qt[j][base : base + 64, half * 512 : (half + 1) * 512],
                                True,
                                True,
                                chain="pe_attn",
                            )
                        ex = workp.tile([P, t2], DT, tag="expT", name="ex", bufs=4)
                        nc.scalar.activation(
                            out=ex,
                            in_=ps_s,
                            func=mybir.ActivationFunctionType.Exp,
                            bias=mask_sb[:, c : c + 1],
                            scale=0.125,
                        )
                        exs.append(ex)
                    if prev is not None:
                        emit_pv(c - 1, prev)
                    prev = exs
                emit_pv(NCH - 1, prev)
                # flush the previous pair first: its bcast matmul runs now
                # (reciprocal long done), and its muls free ps_b slots early.
                while pending:
                    flush_norm()
                # both accumulator copies BEFORE the slow reciprocals: the
                # in-order DVE must release both PSUM slots promptly.
                Us = []
                for ab in range(2):
                    U = workp.tile([65, t2], F32, tag="unorm", name="U", bufs=4)
                    chain_dve(nc.vector.tensor_copy(out=U, in_=ps_os[ab]))
                    Us.append(U)
                for ab, (_h, kvh, base) in enumerate(heads):
                    U = Us[ab]
                    inv = workp.tile([1, t2], DT, tag="inv", name="inv", bufs=3)
                    with nc.allow_low_precision("f32r denom feeds bcast matmul"):
                        chain_dve(nc.vector.reciprocal(out=inv, in_=U[64:65, :]))
                    pending.append((U, inv, j, base))
            while pending:
                flush_norm()

            for s in range(t2 // 128):
                ob = outp.tile([P, D_MODEL], F32, tag="ob", name="ob")
                for n in range(2):
                    ps_f = pp_big.tile([P, 512], F32, tag="big", name="ps_f")
                    for p_ in range(NPAIR):
                        MM(
                            ps_f,
                            attnT[p_][:, s * 128 : (s + 1) * 128],
                            wout_sb[:, p_, n * 512 : (n + 1) * 512],
                            p_ == 0,
                            p_ == NPAIR - 1,
                        )
                    nc.vector.tensor_copy(
                        out=ob[:, n * 512 : (n + 1) * 512], in_=ps_f
                    )
                r0 = it2 * t2 + s * 128
                nc.sync.dma_start(out=out[r0 : r0 + 128, :], in_=ob)

    nc.compile()
    return nc


# ---------------------------------------------------------------------------
# host-side sharding / prep
# ---------------------------------------------------------------------------

_HEAD_PERM = [0, 4, 1, 5, 2, 6, 3, 7]  # local head order inside pair tiles


def _rope_tables(tkv):
    theta = ROPE_BASE ** (-np.arange(0, D_K, 2, dtype=np.float32) / D_K)  # [32]
    pos = np.arange(tkv, dtype=np.float32)[:, None]  # [tkv,1]
    ang = pos * theta[None, :]  # [tkv,32]
    c = np.cos(ang).T.astype(np.float32)  # [32, tkv]
    s = np.sin(ang).T.astype(np.float32)
    cosF = np.concatenate([c, c, c, c], axis=0)
    sinF = np.concatenate([-s, s, -s, s], axis=0)
    return np.ascontiguousarray(cosF), np.ascontiguousarray(sinF)


def make_in_maps(query, key_value, kv_mask, w_q, w_k, w_v, w_out, tq=TQ, tkv=TKV):
    nb = query.shape[0]
    cosF, sinF = _rope_tables(max(tq, tkv))
    cosF = cosF[:, :tkv] if cosF.shape[1] != tkv else cosF
    sinF = sinF[:, :tkv] if sinF.shape[1] != tkv else sinF
    cosQ = cosF  # same tables sliced by column inside the kernel
    del cosQ
    in_maps = []
    col_perm = np.concatenate(
        [np.arange(h * D_K, (h + 1) * D_K) for h in _HEAD_PERM]
    )
    for core in range(2 * nb):
        b = core // 2
        g = core % 2
        qTb = np.ascontiguousarray(query[b].T.astype(np.float32))
        kvTb = np.ascontiguousarray(key_value[b].T.astype(np.float32))
        wq_g = w_q[:, g * 512 : (g + 1) * 512][:, col_perm]
        wk_g = w_k[:, g * 128 : (g + 1) * 128]
        wv_g = w_v[:, g * 128 : (g + 1) * 128]
        wout_g = w_out[g * 512 : (g + 1) * 512, :][col_perm, :]
        maskb = np.where(kv_mask[b], 0.0, NEG_BIAS).astype(np.float32)
        maskb = np.ascontiguousarray(maskb.reshape(tkv // 128, 128).T)
        ones_arr = np.ones((128, 64), np.float32)
        in_maps.append(
            {
                "qT": qTb,
                "kvT": kvTb,
                "wq": np.ascontiguousarray(wq_g.astype(np.float32)),
                "wk": np.ascontiguousarray(wk_g.astype(np.float32)),
                "wv": np.ascontiguousarray(wv_g.astype(np.float32)),
                "wout": np.ascontiguousarray(wout_g.astype(np.float32)),
                "cosF": cosF,
                "sinF": sinF,
                "maskb": maskb,
                "onesc": ones_arr,
            }
        )
    return in_maps


_NC_CACHE = {}


def _get_nc(tq=TQ, tkv=TKV, t2=1024, use_f32r=True):
    key = (tq, tkv, t2, use_f32r)
    if key not in _NC_CACHE:
        _NC_CACHE[key] = build_bass(tq, tkv, t2, use_f32r)
    return _NC_CACHE[key]


def _run(inputs, trace=False):
    query = np.asarray(inputs["query"], dtype=np.float32)
    key_value = np.asarray(inputs["key_value"], dtype=np.float32)
    kv_mask = np.asarray(inputs["kv_mask"])
    w_q = np.asarray(inputs["w_q"], dtype=np.float32)
    w_k = np.asarray(inputs["w_k"], dtype=np.float32)
    w_v = np.asarray(inputs["w_v"], dtype=np.float32)
    w_out = np.asarray(inputs["w_out"], dtype=np.float32)
    nb, tq, _ = query.shape
    tkv = key_value.shape[1]

    nc = _get_nc(tq, tkv)
    in_maps = make_in_maps(query, key_value, kv_mask, w_q, w_k, w_v, w_out, tq, tkv)
    res = run_bass_kernel_spmd(
        nc, in_maps, list(range(2 * nb)), trace=trace, trace_cores=[0]
    )
    outs = [np.asarray(r["out"]) for r in res.results]
    full = np.stack([outs[2 * b] + outs[2 * b + 1] for b in range(nb)])

    query_mask = np.asarray(inputs["query_mask"])
    if not query_mask.all():
        # masked query rows: reference yields uniform attention over all kv
        for b in range(nb):
            rows = ~query_mask[b]
            if rows.any():
                V = key_value[b] @ w_v  # [tkv, 256]
                meanV = V.mean(axis=0)  # [256]
                group = N_HEADS // NUM_KV_HEADS
                feat = np.concatenate([meanV.reshape(NUM_KV_HEADS, D_K)[h // group]
                                       for h in range(N_HEADS)])
                full[b, rows, :] = feat @ w_out
    return full.astype(np.float32), res


def kernel(**inputs):
    out, _ = _run(inputs, trace=False)
    return out


def kernel_traced(**inputs):
    out, res = _run(inputs, trace=True)
    return out, res


if __name__ == "__main__":
    print("kernel.py is a library; use test.py")



# revision 12
# speedup vs baseline: 1.5578x; 1.5578x over previous
"""Cross-attention (GQA + RoPE) Trainium2 Bass kernel.

Sharding: 8 cores = 4 batches x 2 head-groups.
  core i -> batch b = i // 2, head-group g = i % 2
  Each core computes 8 query heads / 2 kv heads of one batch and a
  row-parallel partial of the output projection; the host sums the two
  partials per batch.

Key optimizations over the v1 baseline:
  * kv compaction: ~50% of kv positions are masked out (kv_mask); the host
    gathers only the valid positions (plus padding to a multiple of 128),
    which is mathematically exact for softmax and nearly halves the
    attention-phase work on every engine.
  * bf16 operands end-to-end (weights, activations, probs): halves DMA and
    SBUF footprint, avoids the f32r narrow-matmul penalty; PSUM accumulation
    stays f32.  Measured end-to-end rel err ~6e-3 (gate 2e-2).
  * softmax denominators inverted with reciprocal_approx_fast (single-pass
    custom DVE op, ~5x faster than the exact reciprocal whose 6.5us
    instances dominated the v1 critical path).

Per-core layout (all "T" tensors have head_dim / feature on partitions):
  qT   [1024, TQ]    query^T             (host-transposed, bf16)
  kvT  [1024, TKVC]  compacted key_value^T (bf16)
  wq   [1024, 512]   w_q columns of this head group, head-PERMUTED so that
                     pair-tile j holds local heads (j, j+4) -> rows (0-63,
                     64-127), making Q/K partition bases match per head.
  wk   [1024, 128]   w_k columns (2 kv heads)         (bf16)
  wv   [1024, 128]   w_v columns                      (bf16)
  wout [512, 1024]   w_out rows, same head permutation (bf16)
  kcos/ksin [128, TKVC] rope tables gathered at the kept kv positions (f32)
  qcos/qsin [128, TQ]   rope tables for query positions 0..TQ-1 (f32)
  maskb [128, NCH]   additive bias per 128-chunk: 0 valid / -30000 padding
  onesc [128, 64]    ones (bf16) for the V ones-column; row 0 is also the
                     f32r ones vector of the broadcast matmul (via onesr)

Algorithm per core:
  K^T = rope(wk^T @ kvT)      resident [128, TKVC]  (2 kv heads stacked)
  V   = (kvT chunks)^T @ wv   resident [128, 65*NCH] per kv head, with an
                              appended ones-column per chunk (softmax denom)
  per tq block T2, per head-pair:
     scores^T chunk [kv 128, tq T2] = K_c^T.T @ Q^T   (PSUM)
     e = exp(0.125*scores^T + mask_bias)              (ACT, bf16 out)
     psum_o [65, T2] += V_c_aug.T @ e                 (row 64 = sum of exp)
     inv = approx(1/psum_o[64]); attnT = psum_o[0:64] * bcast(inv)
  out[tq, :] partial = attnT.T @ wout                 (PSUM -> DMA)
"""

import os
from contextlib import ExitStack

import numpy as np

import concourse.bass as bass
import concourse.bacc as bacc
import concourse.mybir as mybir
import concourse.tile as tile
from concourse.bass_utils import run_bass_kernel_spmd

F32 = mybir.dt.float32
R32 = mybir.dt.float32r
BF16 = mybir.dt.bfloat16

D_MODEL = 1024
N_HEADS = 16
NUM_KV_HEADS = 4
D_K = 64
ROPE_BASE = 10000.0
B = 4
TQ = 2048
N_CORES = 8

NEG_BIAS = -30000.0


def _widths(total, w=512):
    """Split total into chunks of width w plus a remainder (multiple of 128)."""
    out = [w] * (total // w)
    if total % w:
        out.append(total % w)
    return out


def build_bass(tq=TQ, tkv=1152, t2=1024):
    """Build the single-core SPMD program (same program on all 8 cores)."""
    nc = bacc.Bacc("TRN2", target_bir_lowering=False, debug=False)
    P = 128
    NCH = tkv // 128          # attention kv chunks
    NT2 = tq // t2            # tq blocks
    NHALF = t2 // 512         # 512-wide matmul slices per tq block
    NPAIR = 4                 # head-pair tiles per core
    DT = BF16

    qT = nc.dram_tensor("qT", [D_MODEL, tq], DT, kind="ExternalInput").ap()
    kvT = nc.dram_tensor("kvT", [D_MODEL, tkv], DT, kind="ExternalInput").ap()
    wq = nc.dram_tensor("wq", [D_MODEL, 512], DT, kind="ExternalInput").ap()
    wk = nc.dram_tensor("wk", [D_MODEL, 128], DT, kind="ExternalInput").ap()
    wv = nc.dram_tensor("wv", [D_MODEL, 128], DT, kind="ExternalInput").ap()
    wout = nc.dram_tensor("wout", [512, D_MODEL], DT, kind="ExternalInput").ap()
    kcos = nc.dram_tensor("kcos", [P, tkv], F32, kind="ExternalInput").ap()
    ksin = nc.dram_tensor("ksin", [P, tkv], F32, kind="ExternalInput").ap()
    qcos = nc.dram_tensor("qcos", [P, tq], F32, kind="ExternalInput").ap()
    qsin = nc.dram_tensor("qsin", [P, tq], F32, kind="ExternalInput").ap()
    maskb = nc.dram_tensor("maskb", [P, NCH], F32, kind="ExternalInput").ap()
    onesc = nc.dram_tensor("onesc", [P, 64], DT, kind="ExternalInput").ap()
    out = nc.dram_tensor("out", [tq, D_MODEL], F32, kind="ExternalOutput").ap()

    with tile.TileContext(nc) as tc, ExitStack() as ctx:
        const = ctx.enter_context(tc.tile_pool(name="const", bufs=1))
        blkp = ctx.enter_context(tc.tile_pool(name="blkp", bufs=2))
        qpool = ctx.enter_context(tc.tile_pool(name="qpool", bufs=1))
        apool = ctx.enter_context(tc.tile_pool(name="apool", bufs=1))
        workp = ctx.enter_context(tc.tile_pool(name="workp", bufs=3))
        ropep = ctx.enter_context(tc.tile_pool(name="ropep", bufs=2))
        outp = ctx.enter_context(tc.tile_pool(name="outp", bufs=2))
        pp_big = ctx.enter_context(tc.tile_pool(name="pp_big", bufs=2, space="PSUM"))
        pp_acc = ctx.enter_context(tc.tile_pool(name="pp_acc", bufs=2, space="PSUM"))

        def MM(out_ap, lhsT, rhs, start, stop, chain=None):
            inst = nc.tensor.matmul(out_ap, lhsT, rhs, start=start, stop=stop)
            if chain is not None:
                tc.chain_iter_dep(chain, inst.ins)
            return inst

        def chain_dve(inst):
            tc.chain_iter_dep("dve_norm", inst.ins)
            return inst

        # ---- constants / weights -------------------------------------------------
        wk_sb = const.tile([P, 8, 128], DT)
        nc.gpsimd.dma_start(out=wk_sb, in_=wk.rearrange("(c p) f -> p c f", p=P))
        wv_sb = const.tile([P, 8, 128], DT)
        nc.gpsimd.dma_start(out=wv_sb, in_=wv.rearrange("(c p) f -> p c f", p=P))
        kcos_sb = const.tile([P, tkv], F32)
        nc.gpsimd.dma_start(out=kcos_sb, in_=kcos)
        ksin_sb = const.tile([P, tkv], F32)
        nc.gpsimd.dma_start(out=ksin_sb, in_=ksin)
        wq_sb = const.tile([P, 8, 512], DT)
        nc.gpsimd.dma_start(out=wq_sb, in_=wq.rearrange("(c p) f -> p c f", p=P))
        qcos_sb = const.tile([P, tq], F32)
        nc.gpsimd.dma_start(out=qcos_sb, in_=qcos)
        qsin_sb = const.tile([P, tq], F32)
        nc.gpsimd.dma_start(out=qsin_sb, in_=qsin)
        wout_sb = const.tile([P, 4, D_MODEL], DT)
        nc.gpsimd.dma_start(out=wout_sb, in_=wout.rearrange("(c p) f -> p c f", p=P))
        mask_sb = const.tile([P, NCH], F32)
        nc.gpsimd.dma_start(out=mask_sb, in_=maskb)

        Kt = const.tile([P, tkv], DT)
        Vt = [const.tile([P, NCH * 65], DT, name=f"Vt{i}") for i in range(2)]
        for i in range(2):
            nc.gpsimd.dma_start(
                out=Vt[i].rearrange("p (c k) -> p c k", k=65)[:, :, 64],
                in_=onesc[:, :NCH],
            )

        def rope_apply(dest, ps, cos_sb, sin_sb, col0, width):
            """dest[128, width] (SBUF) = rope(ps[128, width] PSUM), table
            cols col0..col0+width. Rows: two stacked heads, each [x1;x2]."""
            cs = cos_sb[:, col0 : col0 + width]
            t_cos = ropep.tile([P, t2], F32, tag="rope", name="t_cos")
            t_u = ropep.tile([P, t2], F32, tag="rope", name="t_u")
            tc_ = t_cos[:, :width]
            tu_ = t_u[:, :width]
            nc.vector.tensor_mul(tc_, ps, cs)
            for b0 in (0, 64):
                # sin rows [b0:b0+32] = -sin, [b0+32:b0+64] = +sin
                nc.vector.tensor_mul(
                    tu_[b0 : b0 + 32, :],
                    ps[b0 + 32 : b0 + 64, :],
                    sin_sb[b0 : b0 + 32, col0 : col0 + width],
                )
                nc.vector.tensor_mul(
                    tu_[b0 + 32 : b0 + 64, :],
                    ps[b0 : b0 + 32, :],
                    sin_sb[b0 + 32 : b0 + 64, col0 : col0 + width],
                )
            nc.vector.tensor_add(dest, tc_, tu_)

        # ---- phase KV: K/V projections ------------------------------------------
        col0 = 0
        for w in _widths(tkv):
            kv_blk = blkp.tile([P, 8, 512], DT, tag="blk", name="kv_blk")
            kvb = kv_blk[:, :, :w]
            nc.gpsimd.dma_start(
                out=kvb,
                in_=kvT.rearrange("(c p) t -> p c t", p=P)[:, :, col0 : col0 + w],
            )
            ps_k = pp_big.tile([P, t2], F32, tag="big", name="ps_k")
            pk = ps_k[:, :w]
            for d in range(8):
                MM(pk, wk_sb[:, d, :], kvb[:, d, :], d == 0, d == 7)
            rope_apply(Kt[:, col0 : col0 + w], pk, kcos_sb, ksin_sb, col0, w)
            for s in range(w // 128):
                ps_v = pp_big.tile([P, t2], F32, tag="big", name="ps_v")
                pv = ps_v[:, 0:128]
                for d in range(8):
                    MM(
                        pv,
                        kvb[:, d, s * 128 : (s + 1) * 128],
                        wv_sb[:, d, :],
                        d == 0,
                        d == 7,
                    )
                c = col0 // 128 + s
                nc.vector.tensor_copy(
                    out=Vt[0][:, c * 65 : c * 65 + 64], in_=pv[:, 0:64]
                )
                nc.vector.tensor_copy(
                    out=Vt[1][:, c * 65 : c * 65 + 64], in_=pv[:, 64:128]
                )
            col0 += w

        # ---- per tq block: Q proj -> attention -> output projection -------------
        for it2 in range(NT2):
            q_blks = []
            for half in range(NHALF):
                qb = blkp.tile([P, 8, 512], DT, tag="blk", name="q_blk")
                c0 = it2 * t2 + half * 512
                nc.gpsimd.dma_start(
                    out=qb,
                    in_=qT.rearrange("(c p) t -> p c t", p=P)[:, :, c0 : c0 + 512],
                )
                q_blks.append(qb)

            Qt = []
            for j in range(NPAIR):
                ps_q = pp_big.tile([P, t2], F32, tag="big", name="ps_q")
                for half in range(NHALF):
                    for d in range(8):
                        MM(
                            ps_q[:, half * 512 : (half + 1) * 512],
                            wq_sb[:, d, j * 128 : (j + 1) * 128],
                            q_blks[half][:, d, :],
                            d == 0,
                            d == 7,
                        )
                qt = qpool.tile([P, t2], DT, tag=f"Q{j}", name=f"Qt{j}")
                rope_apply(qt, ps_q, qcos_sb, qsin_sb, it2 * t2, t2)
                Qt.append(qt)

            attnT = [
                apool.tile([P, t2], DT, tag=f"A{j}", name=f"attnT{j}")
                for j in range(NPAIR)
            ]

            # normalization of head h is EMITTED after head h+1's attention
            # matmuls so the slow steps (reciprocal, gpsimd broadcast) are
            # off the PE critical path.
            pending = []

            def flush_norm():
                if not pending:
                    return
                U, inv, j_, base_ = pending.pop(0)
                invb = workp.tile([64, t2], F32, tag="invb", name="invb", bufs=2)
                nc.gpsimd.partition_broadcast(invb, inv)
                chain_dve(
                    nc.vector.tensor_mul(
                        attnT[j_][base_ : base_ + 64, :], U[0:64, :], invb
                    )
                )

            # two heads (j, j+4) interleave: while one head's exp is on the
            # Scalar engine, the PE runs the other head's matmuls back-to-back.
            for j in range(NPAIR):
                heads = [(j, 0, 0), (j + 4, 1, 64)]  # (head, kvh, base)
                ps_os = [
                    pp_acc.tile([65, t2], F32, tag="acc", name=f"ps_o{ab}")
                    for ab in range(2)
                ]

                def emit_pv(c_, exs_):
                    for ab, (_h, kvh, _base) in enumerate(heads):
                        for half in range(NHALF):
                            MM(
                                ps_os[ab][:, half * 512 : (half + 1) * 512],
                                Vt[kvh][:, c_ * 65 : c_ * 65 + 65],
                                exs_[ab][:, half * 512 : (half + 1) * 512],
                                c_ == 0,
                                c_ == NCH - 1,
                                chain="pe_attn",
                            )

                # PV lags the scores by one chunk so no PE instruction ever
                # reaches the queue head with an unresolved wait.
                prev = None
                for c in range(NCH):
                    exs = []
                    for ab, (_h, kvh, base) in enumerate(heads):
                        ps_s = pp_big.tile([P, t2], F32, tag="big", name="ps_s")
                        for half in range(NHALF):
                            MM(
                                ps_s[:, half * 512 : (half + 1) * 512],
                                Kt[base : base + 64, c * 128 : (c + 1) * 128],
                                Qt[j][base : base + 64, half * 512 : (half + 1) * 512],
                                True,
                                True,
                                chain="pe_attn",
                            )
                        ex = workp.tile([P, t2], DT, tag="expT", name="ex", bufs=4)
                        nc.scalar.activation(
                            out=ex,
                            in_=ps_s,
                            func=mybir.ActivationFunctionType.Exp,
                            bias=mask_sb[:, c : c + 1],
                            scale=0.125,
                        )
                        exs.append(ex)
                    if prev is not None:
                        emit_pv(c - 1, prev)
                    prev = exs
                emit_pv(NCH - 1, prev)
                # flush the previous pair first: its bcast matmul runs now
                # (reciprocal long done), and its muls free ps_b slots early.
                while pending:
                    flush_norm()
                # both accumulator copies BEFORE the reciprocals: the
                # in-order DVE must release both PSUM slots promptly.
                Us = []
                dens = []
                for ab in range(2):
                    U = workp.tile([65, t2], F32, tag="unorm", name="U", bufs=4)
                    chain_dve(nc.vector.tensor_copy(out=U, in_=ps_os[ab]))
                    Us.append(U)
                    # denominator row -> partition 0 via ACT Copy (present in
                    # every ACT table, so no table-load thrash with Exp); the
                    # custom-DVE reciprocal needs a base-partition-0 input.
                    den = workp.tile([1, t2], F32, tag="den", name="den", bufs=2)
                    nc.scalar.copy(out=den, in_=ps_os[ab][64:65, :])
                    dens.append(den)
                for ab, (_h, kvh, base) in enumerate(heads):
                    U = Us[ab]
                    inv = workp.tile([1, t2], F32, tag="inv", name="inv", bufs=2)
                    chain_dve(
                        nc.vector.reciprocal_approx_fast(out=inv, in_=dens[ab])
                    )
                    pending.append((U, inv, j, base))
            while pending:
                flush_norm()

            for s in range(t2 // 128):
                ob = outp.tile([P, D_MODEL], F32, tag="ob", name="ob")
                for n in range(2):
                    ps_f = pp_big.tile([P, t2], F32, tag="big", name="ps_f")
                    psf = ps_f[:, 0:512]
                    for p_ in range(NPAIR):
                        MM(
                            psf,
                            attnT[p_][:, s * 128 : (s + 1) * 128],
                            wout_sb[:, p_, n * 512 : (n + 1) * 512],
                            p_ == 0,
                            p_ == NPAIR - 1,
                        )
                    nc.vector.tensor_copy(
                        out=ob[:, n * 512 : (n + 1) * 512], in_=psf
                    )
                r0 = it2 * t2 + s * 128
                nc.sync.dma_start(out=out[r0 : r0 + 128, :], in_=ob)

    nc.compile()
    return nc


# ---------------------------------------------------------------------------
# host-side sharding / prep
# ---------------------------------------------------------------------------

_HEAD_PERM = [0, 4, 1, 5, 2, 6, 3, 7]  # local head order inside pair tiles

try:
    import ml_dtypes

    _BF16 = ml_dtypes.bfloat16
except ImportError:  # pragma: no cover
    import jax.numpy as jnp

    _BF16 = jnp.bfloat16


def _bf(x):
    return np.ascontiguousarray(np.asarray(x, dtype=np.float32).astype(_BF16))


def _rope_tables(positions):
    """cos/sin tables [128, len(positions)] stacked for two heads."""
    theta = ROPE_BASE ** (-np.arange(0, D_K, 2, dtype=np.float32) / D_K)  # [32]
    pos = np.asarray(positions, dtype=np.float32)[:, None]  # [T,1]
    ang = pos * theta[None, :]  # [T,32]
    c = np.cos(ang).T.astype(np.float32)  # [32, T]
    s = np.sin(ang).T.astype(np.float32)
    cosF = np.concatenate([c, c, c, c], axis=0)
    sinF = np.concatenate([-s, s, -s, s], axis=0)
    return np.ascontiguousarray(cosF), np.ascontiguousarray(sinF)


def make_in_maps(query, key_value, kv_mask, w_q, w_k, w_v, w_out, tq, tkv_c):
    nb = query.shape[0]
    qcos, qsin = _rope_tables(np.arange(tq))
    in_maps = []
    col_perm = np.concatenate(
        [np.arange(h * D_K, (h + 1) * D_K) for h in _HEAD_PERM]
    )
    for core in range(2 * nb):
        b = core // 2
        g = core % 2
        idx = np.nonzero(kv_mask[b])[0]
        nv = len(idx)
        kv_c = np.zeros((tkv_c, key_value.shape[2]), np.float32)
        kv_c[:nv] = key_value[b][idx]
        pos = np.zeros(tkv_c, np.int64)
        pos[:nv] = idx
        kcos, ksin = _rope_tables(pos)
        maskb = np.full(tkv_c, NEG_BIAS, np.float32)
        maskb[:nv] = 0.0
        maskb = np.ascontiguousarray(maskb.reshape(tkv_c // 128, 128).T)

        qTb = np.ascontiguousarray(_bf(query[b]).T)
        kvTb = np.ascontiguousarray(_bf(kv_c).T)
        wq_g = w_q[:, g * 512 : (g + 1) * 512][:, col_perm]
        wk_g = w_k[:, g * 128 : (g + 1) * 128]
        wv_g = w_v[:, g * 128 : (g + 1) * 128]
        wout_g = w_out[g * 512 : (g + 1) * 512, :][col_perm, :]
        in_maps.append(
            {
                "qT": qTb,
                "kvT": kvTb,
                "wq": _bf(wq_g),
                "wk": _bf(wk_g),
                "wv": _bf(wv_g),
                "wout": _bf(wout_g),
                "kcos": kcos,
                "ksin": ksin,
                "qcos": qcos,
                "qsin": qsin,
                "maskb": maskb,
                "onesc": _bf(np.ones((128, 64), np.float32)),
            }
        )
    return in_maps


_NC_CACHE = {}


def _get_nc(tq, tkv_c, t2=1024):
    key = (tq, tkv_c, t2)
    if key not in _NC_CACHE:
        _NC_CACHE[key] = build_bass(tq, tkv_c, t2)
    return _NC_CACHE[key]


def _run(inputs, trace=False):
    query = np.asarray(inputs["query"], dtype=np.float32)
    key_value = np.asarray(inputs["key_value"], dtype=np.float32)
    kv_mask = np.asarray(inputs["kv_mask"])
    w_q = np.asarray(inputs["w_q"], dtype=np.float32)
    w_k = np.asarray(inputs["w_k"], dtype=np.float32)
    w_v = np.asarray(inputs["w_v"], dtype=np.float32)
    w_out = np.asarray(inputs["w_out"], dtype=np.float32)
    nb, tq, _ = query.shape

    max_valid = int(kv_mask.sum(axis=1).max())
    tkv_c = max(256, -(-max_valid // 128) * 128)

    nc = _get_nc(tq, tkv_c)
    in_maps = make_in_maps(
        query, key_value, kv_mask, w_q, w_k, w_v, w_out, tq, tkv_c
    )
    res = run_bass_kernel_spmd(
        nc, in_maps, list(range(2 * nb)), trace=trace, trace_cores=[0]
    )
    outs = [np.asarray(r["out"]) for r in res.results]
    full = np.stack([outs[2 * b] + outs[2 * b + 1] for b in range(nb)])

    query_mask = np.asarray(inputs["query_mask"])
    if not query_mask.all():
        # masked query rows: reference yields uniform attention over all kv
        for b in range(nb):
            rows = ~query_mask[b]
            if rows.any():
                V = key_value[b] @ w_v  # [tkv, 256]
                meanV = V.mean(axis=0)  # [256]
                group = N_HEADS // NUM_KV_HEADS
                feat = np.concatenate(
                    [
                        meanV.reshape(NUM_KV_HEADS, D_K)[h // group]
                        for h in range(N_HEADS)
                    ]
                )
                full[b, rows, :] = feat @ w_out
    return full.astype(np.float32), res


def kernel(**inputs):
    out, _ = _run(inputs, trace=False)
    return out


def kernel_traced(**inputs):
    out, res = _run(inputs, trace=True)
    return out, res


if __name__ == "__main__":
    print("kernel.py is a library; use test.py")


# revision 13
# speedup vs baseline: 1.7260x; 1.1080x over previous
"""Cross-attention (GQA + RoPE) Trainium2 Bass kernel.

Sharding: 8 cores = 4 batches x 2 head-groups.
  core i -> batch b = i // 2, head-group g = i % 2
  Each core computes 8 query heads / 2 kv heads of one batch and a
  row-parallel partial of the output projection; the host sums the two
  partials per batch.

Key optimizations over the v1 baseline (591us):
  * kv compaction: ~50% of kv positions are masked out (kv_mask); the host
    gathers only the valid positions (plus padding to a multiple of 128),
    which is mathematically exact for softmax and nearly halves the
    attention-phase work on every engine.
  * bf16 operands end-to-end (weights, activations, probs): halves DMA and
    SBUF footprint, avoids the f32r narrow-matmul penalty; PSUM accumulation
    stays f32.  Measured end-to-end rel err ~6e-3 (gate 2e-2).
  * softmax denominators inverted with reciprocal_approx_fast (single custom
    DVE op, ~1.2us vs 6.5us for the exact reciprocal).  The custom op only
    works on base-partition-0 inputs, so the denominator row is first moved
    from PSUM partition 64 with an ACT Copy (Copy lives in every ACT table,
    so no table-load thrash against Exp).  The inverse is partition-broadcast
    on GpSimd, off the busy engines.
  * software pipelining: the attention inner loop (scores -> exp -> PV) is
    ACT-bound per chunk, which used to stall the PE every chunk and drop it
    to the mid p-state clock (2x slower matmuls).  Now the Q projection of
    the NEXT head pair and the output projection of the PREVIOUS tq block are
    emitted as filler matmuls inside the attention chunk loop, keeping the PE
    continuously busy at the peak p-state.

Per-core layout (all "T" tensors have head_dim / feature on partitions):
  qT   [1024, TQ]    query^T             (host-transposed, bf16)
  kvT  [1024, TKVC]  compacted key_value^T (bf16)
  wq   [1024, 512]   w_q columns of this head group, head-PERMUTED so that
                     pair-tile j holds local heads (j, j+4) -> rows (0-63,
                     64-127), making Q/K partition bases match per head.
  wk   [1024, 128]   w_k columns (2 kv heads)         (bf16)
  wv   [1024, 128]   w_v columns                      (bf16)
  wout [512, 1024]   w_out rows, same head permutation (bf16)
  kcos/ksin [128, TKVC] rope tables gathered at the kept kv positions (f32)
  qcos/qsin [128, TQ]   rope tables for query positions 0..TQ-1 (f32)
  maskb [128, NCH]   additive bias per 128-chunk: 0 valid / -30000 padding
  onesc [128, 64]    ones (bf16) for the V ones-column

PSUM map (exactly 8 banks):
  sc   2 x [128, t2] f32 (4 banks) - scores ring, consumed by exp
  fill 2 x [128, 512] f32 (2 banks) - K/V/Q projections + out projection
  acc  1 x [65, t2]  f32 (2 banks) - PV accumulator (row 64 = denominator)
"""

import os
from collections import deque
from contextlib import ExitStack

import numpy as np

import concourse.bass as bass
import concourse.bacc as bacc
import concourse.mybir as mybir
import concourse.tile as tile
from concourse.bass_utils import run_bass_kernel_spmd

F32 = mybir.dt.float32
BF16 = mybir.dt.bfloat16

D_MODEL = 1024
N_HEADS = 16
NUM_KV_HEADS = 4
D_K = 64
ROPE_BASE = 10000.0
B = 4
TQ = 2048
N_CORES = 8

NEG_BIAS = -30000.0


def _widths(total, w=512):
    out = [w] * (total // w)
    if total % w:
        out.append(total % w)
    return out


def build_bass(tq=TQ, tkv=1152, t2=1024):
    """Build the single-core SPMD program (same program on all 8 cores)."""
    nc = bacc.Bacc("TRN2", target_bir_lowering=False, debug=False)
    P = 128
    NCH = tkv // 128          # attention kv chunks
    NT2 = tq // t2            # tq blocks
    NHALF = t2 // 512         # 512-wide slices per tq block
    NPAIR = 4                 # head-pair tiles per core
    DT = BF16

    qT = nc.dram_tensor("qT", [D_MODEL, tq], DT, kind="ExternalInput").ap()
    kvT = nc.dram_tensor("kvT", [D_MODEL, tkv], DT, kind="ExternalInput").ap()
    wq = nc.dram_tensor("wq", [D_MODEL, 512], DT, kind="ExternalInput").ap()
    wk = nc.dram_tensor("wk", [D_MODEL, 128], DT, kind="ExternalInput").ap()
    wv = nc.dram_tensor("wv", [D_MODEL, 128], DT, kind="ExternalInput").ap()
    wout = nc.dram_tensor("wout", [512, D_MODEL], DT, kind="ExternalInput").ap()
    kcos = nc.dram_tensor("kcos", [P, tkv], F32, kind="ExternalInput").ap()
    ksin = nc.dram_tensor("ksin", [P, tkv], F32, kind="ExternalInput").ap()
    qcos = nc.dram_tensor("qcos", [P, tq], F32, kind="ExternalInput").ap()
    qsin = nc.dram_tensor("qsin", [P, tq], F32, kind="ExternalInput").ap()
    maskb = nc.dram_tensor("maskb", [P, NCH], F32, kind="ExternalInput").ap()
    onesc = nc.dram_tensor("onesc", [P, 64], DT, kind="ExternalInput").ap()
    out = nc.dram_tensor("out", [tq, D_MODEL], F32, kind="ExternalOutput").ap()

    with tile.TileContext(nc) as tc, ExitStack() as ctx:
        const = ctx.enter_context(tc.tile_pool(name="const", bufs=1))
        qpool = ctx.enter_context(tc.tile_pool(name="qpool", bufs=2))
        apool = ctx.enter_context(tc.tile_pool(name="apool", bufs=2))
        workp = ctx.enter_context(tc.tile_pool(name="workp", bufs=2))
        ropep = ctx.enter_context(tc.tile_pool(name="ropep", bufs=2))
        outp = ctx.enter_context(tc.tile_pool(name="outp", bufs=2))
        pp_sc = ctx.enter_context(tc.tile_pool(name="pp_sc", bufs=2, space="PSUM"))
        pp_fill = ctx.enter_context(tc.tile_pool(name="pp_fill", bufs=2, space="PSUM"))
        pp_acc = ctx.enter_context(tc.tile_pool(name="pp_acc", bufs=1, space="PSUM"))

        def MM(out_ap, lhsT, rhs, start, stop):
            return nc.tensor.matmul(out_ap, lhsT, rhs, start=start, stop=stop)

        # ---- constants / weights -------------------------------------------------
        wk_sb = const.tile([P, 8, 128], DT)
        nc.gpsimd.dma_start(out=wk_sb, in_=wk.rearrange("(c p) f -> p c f", p=P))
        wv_sb = const.tile([P, 8, 128], DT)
        nc.gpsimd.dma_start(out=wv_sb, in_=wv.rearrange("(c p) f -> p c f", p=P))
        kvT_sb = const.tile([P, 8, tkv], DT)
        nc.gpsimd.dma_start(out=kvT_sb, in_=kvT.rearrange("(c p) t -> p c t", p=P))
        kcos_sb = const.tile([P, tkv], F32)
        nc.gpsimd.dma_start(out=kcos_sb, in_=kcos)
        ksin_sb = const.tile([P, tkv], F32)
        nc.gpsimd.dma_start(out=ksin_sb, in_=ksin)
        wq_sb = const.tile([P, 8, 512], DT)
        nc.gpsimd.dma_start(out=wq_sb, in_=wq.rearrange("(c p) f -> p c f", p=P))
        qT_sb = const.tile([P, 8, tq], DT)
        nc.gpsimd.dma_start(out=qT_sb, in_=qT.rearrange("(c p) t -> p c t", p=P))
        qcos_sb = const.tile([P, tq], F32)
        nc.gpsimd.dma_start(out=qcos_sb, in_=qcos)
        qsin_sb = const.tile([P, tq], F32)
        nc.gpsimd.dma_start(out=qsin_sb, in_=qsin)
        wout_sb = const.tile([P, 4, D_MODEL], DT)
        nc.gpsimd.dma_start(out=wout_sb, in_=wout.rearrange("(c p) f -> p c f", p=P))
        mask_sb = const.tile([P, NCH], F32)
        nc.gpsimd.dma_start(out=mask_sb, in_=maskb)

        Kt = const.tile([P, tkv], DT)
        Vt = [const.tile([P, NCH * 65], DT, name=f"Vt{i}") for i in range(2)]
        for i in range(2):
            nc.gpsimd.dma_start(
                out=Vt[i].rearrange("p (c k) -> p c k", k=65)[:, :, 64],
                in_=onesc[:, :NCH],
            )

        def rope_apply(dest, ps, cos_sb, sin_sb, col0, width):
            """dest[128, width] (SBUF) = rope(ps[128, width] PSUM), table
            cols col0..col0+width. Rows: two stacked heads, each [x1;x2]."""
            cs = cos_sb[:, col0 : col0 + width]
            t_cos = ropep.tile([P, 512], F32, tag="rope", name="t_cos")
            t_u = ropep.tile([P, 512], F32, tag="rope", name="t_u")
            tc_ = t_cos[:, :width]
            tu_ = t_u[:, :width]
            nc.vector.tensor_mul(tc_, ps, cs)
            for b0 in (0, 64):
                # sin rows [b0:b0+32] = -sin, [b0+32:b0+64] = +sin
                nc.vector.tensor_mul(
                    tu_[b0 : b0 + 32, :],
                    ps[b0 + 32 : b0 + 64, :],
                    sin_sb[b0 : b0 + 32, col0 : col0 + width],
                )
                nc.vector.tensor_mul(
                    tu_[b0 + 32 : b0 + 64, :],
                    ps[b0 : b0 + 32, :],
                    sin_sb[b0 + 32 : b0 + 64, col0 : col0 + width],
                )
            nc.vector.tensor_add(dest, tc_, tu_)

        # ---- phase KV: K/V projections ------------------------------------------
        col0 = 0
        for w in _widths(tkv):
            ps_k = pp_fill.tile([P, 512], F32, tag="fill", name="ps_k")
            pk = ps_k[:, :w]
            for d in range(8):
                MM(pk, wk_sb[:, d, :], kvT_sb[:, d, col0 : col0 + w], d == 0, d == 7)
            rope_apply(Kt[:, col0 : col0 + w], pk, kcos_sb, ksin_sb, col0, w)
            for s in range(w // 128):
                ps_v = pp_fill.tile([P, 512], F32, tag="fill", name="ps_v")
                pv = ps_v[:, 0:128]
                k0 = col0 + s * 128
                for d in range(8):
                    MM(pv, kvT_sb[:, d, k0 : k0 + 128], wv_sb[:, d, :], d == 0, d == 7)
                c = k0 // 128
                nc.vector.tensor_copy(
                    out=Vt[0][:, c * 65 : c * 65 + 64], in_=pv[:, 0:64]
                )
                nc.vector.tensor_copy(
                    out=Vt[1][:, c * 65 : c * 65 + 64], in_=pv[:, 64:128]
                )
            col0 += w

        # ---- filler machinery ----------------------------------------------------
        fillers = deque()

        def pump(n=1):
            for _ in range(n):
                if fillers:
                    fillers.popleft()()

        def drain():
            while fillers:
                fillers.popleft()()

        Qts = {}      # (j, it2) -> Qt tile
        attnTs = {}   # (j, it2) -> attnT tile

        def push_qproj(j, it2):
            """Emit Q projection + rope for pair j / block it2 as 4 fillers."""
            qt = qpool.tile([P, t2], DT, tag=f"Q{j}", name=f"Qt{j}")
            Qts[(j, it2)] = qt

            def half_closure(half):
                def f():
                    ps = pp_fill.tile([P, 512], F32, tag="fill", name="ps_qh")
                    c0 = it2 * t2 + half * 512
                    for d in range(8):
                        MM(
                            ps,
                            wq_sb[:, d, j * 128 : (j + 1) * 128],
                            qT_sb[:, d, c0 : c0 + 512],
                            d == 0,
                            d == 7,
                        )
                    rope_apply(
                        qt[:, half * 512 : (half + 1) * 512],
                        ps,
                        qcos_sb,
                        qsin_sb,
                        it2 * t2 + half * 512,
                        512,
                    )

                return f

            for half in range(NHALF):
                fillers.append(half_closure(half))

        def push_outproj(it2):
            """Emit output projection of block it2 as per-s-chunk fillers."""
            ats = [attnTs[(j, it2)] for j in range(NPAIR)]

            def s_closure(s, n):
                def f():
                    ps_f = pp_fill.tile([P, 512], F32, tag="fill", name="ps_f")
                    for p_ in range(NPAIR):
                        MM(
                            ps_f,
                            ats[p_][:, s * 128 : (s + 1) * 128],
                            wout_sb[:, p_, n * 512 : (n + 1) * 512],
                            p_ == 0,
                            p_ == NPAIR - 1,
                        )
                    ob = obs[s]
                    nc.vector.tensor_copy(
                        out=ob[:, n * 512 : (n + 1) * 512], in_=ps_f
                    )
                    if n == 1:
                        r0 = it2 * t2 + s * 128
                        nc.sync.dma_start(out=out[r0 : r0 + 128, :], in_=ob)

                return f

            obs = {}
            for s in range(t2 // 128):
                obs[s] = outp.tile([P, D_MODEL], F32, tag="ob", name="ob", bufs=3)
                for n in range(2):
                    fillers.append(s_closure(s, n))

        def push_norm(U, den, j, it2, base):
            """Deferred normalization: reciprocal -> broadcast -> scale."""
            inv = workp.tile([1, t2], F32, tag="inv", name="inv", bufs=2)
            invb = workp.tile([64, t2], F32, tag="invb", name="invb", bufs=2)

            def f1():
                nc.vector.reciprocal_approx_fast(out=inv, in_=den)
                nc.gpsimd.partition_broadcast(invb, inv)

            def f2():
                nc.vector.tensor_mul(
                    attnTs[(j, it2)][base : base + 64, :], U[0:64, :], invb
                )

            fillers.append(f1)
            fillers.append(f2)

        # ---- attention -----------------------------------------------------------
        # first pair of block 0 is projected inline (nothing to overlap with)
        push_qproj(0, 0)
        drain()

        for it2 in range(NT2):
            for j in range(NPAIR):
                attnTs[(j, it2)] = apool.tile(
                    [P, t2], DT, tag=f"A{j}", name=f"attnT{j}"
                )
                # queue next pair's Q projection as filler work
                if j + 1 < NPAIR:
                    push_qproj(j + 1, it2)
                elif it2 + 1 < NT2:
                    push_qproj(0, it2 + 1)
                # queue previous block's output projection (pairs 1..3)
                if it2 > 0 and j == 1:
                    push_outproj(it2 - 1)

                qt = Qts[(j, it2)]
                for ab, (kvh, base) in enumerate([(0, 0), (1, 64)]):
                    ps_o = pp_acc.tile([65, t2], F32, tag="acc", name="ps_o")
                    prev = None
                    for c in range(NCH):
                        ps_s = pp_sc.tile([P, t2], F32, tag="sc", name="ps_s")
                        for half in range(NHALF):
                            MM(
                                ps_s[:, half * 512 : (half + 1) * 512],
                                Kt[base : base + 64, c * 128 : (c + 1) * 128],
                                qt[base : base + 64, half * 512 : (half + 1) * 512],
                                True,
                                True,
                            )
                        ex = workp.tile([P, t2], DT, tag="expT", name="ex", bufs=4)
                        nc.scalar.activation(
                            out=ex,
                            in_=ps_s,
                            func=mybir.ActivationFunctionType.Exp,
                            bias=mask_sb[:, c : c + 1],
                            scale=0.125,
                        )
                        pump(1)
                        if prev is not None:
                            for half in range(NHALF):
                                MM(
                                    ps_o[:, half * 512 : (half + 1) * 512],
                                    Vt[kvh][:, (c - 1) * 65 : (c - 1) * 65 + 65],
                                    prev[:, half * 512 : (half + 1) * 512],
                                    c - 1 == 0,
                                    c - 1 == NCH - 1,
                                )
                        else:
                            pump(1)
                        prev = ex
                    for half in range(NHALF):
                        MM(
                            ps_o[:, half * 512 : (half + 1) * 512],
                            Vt[kvh][:, (NCH - 1) * 65 : (NCH - 1) * 65 + 65],
                            prev[:, half * 512 : (half + 1) * 512],
                            NCH - 1 == 0,
                            True,
                        )
                    # free the accumulator promptly: U copy on DVE, the
                    # denominator row to partition 0 via ACT Copy (no table
                    # load; reciprocal_approx_fast needs base partition 0).
                    U = workp.tile([65, t2], F32, tag="unorm", name="U", bufs=3)
                    nc.vector.tensor_copy(out=U, in_=ps_o)
                    den = workp.tile([1, t2], F32, tag="den", name="den", bufs=2)
                    nc.scalar.copy(out=den, in_=ps_o[64:65, :])
                    push_norm(U, den, j, it2, base)
            drain()

        # ---- tail: output projection of the last block ---------------------------
        push_outproj(NT2 - 1)
        drain()

    nc.compile()
    return nc


# ---------------------------------------------------------------------------
# host-side sharding / prep
# ---------------------------------------------------------------------------

_HEAD_PERM = [0, 4, 1, 5, 2, 6, 3, 7]  # local head order inside pair tiles

try:
    import ml_dtypes

    _BF16 = ml_dtypes.bfloat16
except ImportError:  # pragma: no cover
    import jax.numpy as jnp

    _BF16 = jnp.bfloat16


def _bf(x):
    return np.ascontiguousarray(np.asarray(x, dtype=np.float32).astype(_BF16))


def _rope_tables(positions):
    """cos/sin tables [128, len(positions)] stacked for two heads."""
    theta = ROPE_BASE ** (-np.arange(0, D_K, 2, dtype=np.float32) / D_K)  # [32]
    pos = np.asarray(positions, dtype=np.float32)[:, None]  # [T,1]
    ang = pos * theta[None, :]  # [T,32]
    c = np.cos(ang).T.astype(np.float32)  # [32, T]
    s = np.sin(ang).T.astype(np.float32)
    cosF = np.concatenate([c, c, c, c], axis=0)
    sinF = np.concatenate([-s, s, -s, s], axis=0)
    return np.ascontiguousarray(cosF), np.ascontiguousarray(sinF)


def make_in_maps(query, key_value, kv_mask, w_q, w_k, w_v, w_out, tq, tkv_c):
    nb = query.shape[0]
    qcos, qsin = _rope_tables(np.arange(tq))
    in_maps = []
    col_perm = np.concatenate(
        [np.arange(h * D_K, (h + 1) * D_K) for h in _HEAD_PERM]
    )
    for core in range(2 * nb):
        b = core // 2
        g = core % 2
        idx = np.nonzero(kv_mask[b])[0]
        nv = len(idx)
        kv_c = np.zeros((tkv_c, key_value.shape[2]), np.float32)
        kv_c[:nv] = key_value[b][idx]
        pos = np.zeros(tkv_c, np.int64)
        pos[:nv] = idx
        kcos, ksin = _rope_tables(pos)
        maskb = np.full(tkv_c, NEG_BIAS, np.float32)
        maskb[:nv] = 0.0
        maskb = np.ascontiguousarray(maskb.reshape(tkv_c // 128, 128).T)

        qTb = np.ascontiguousarray(_bf(query[b]).T)
        kvTb = np.ascontiguousarray(_bf(kv_c).T)
        wq_g = w_q[:, g * 512 : (g + 1) * 512][:, col_perm]
        wk_g = w_k[:, g * 128 : (g + 1) * 128]
        wv_g = w_v[:, g * 128 : (g + 1) * 128]
        wout_g = w_out[g * 512 : (g + 1) * 512, :][col_perm, :]
        in_maps.append(
            {
                "qT": qTb,
                "kvT": kvTb,
                "wq": _bf(wq_g),
                "wk": _bf(wk_g),
                "wv": _bf(wv_g),
                "wout": _bf(wout_g),
                "kcos": kcos,
                "ksin": ksin,
                "qcos": qcos,
                "qsin": qsin,
                "maskb": maskb,
                "onesc": _bf(np.ones((128, 64), np.float32)),
            }
        )
    return in_maps


_NC_CACHE = {}


def _get_nc(tq, tkv_c, t2=1024):
    key = (tq, tkv_c, t2)
    if key not in _NC_CACHE:
        _NC_CACHE[key] = build_bass(tq, tkv_c, t2)
    return _NC_CACHE[key]


def _run(inputs, trace=False):
    query = np.asarray(inputs["query"], dtype=np.float32)
    key_value = np.asarray(inputs["key_value"], dtype=np.float32)
    kv_mask = np.asarray(inputs["kv_mask"])
    w_q = np.asarray(inputs["w_q"], dtype=np.float32)
    w_k = np.asarray(inputs["w_k"], dtype=np.float32)
    w_v = np.asarray(inputs["w_v"], dtype=np.float32)
    w_out = np.asarray(inputs["w_out"], dtype=np.float32)
    nb, tq, _ = query.shape

    max_valid = int(kv_mask.sum(axis=1).max())
    tkv_c = max(256, -(-max_valid // 128) * 128)

    nc = _get_nc(tq, tkv_c)
    in_maps = make_in_maps(
        query, key_value, kv_mask, w_q, w_k, w_v, w_out, tq, tkv_c
    )
    res = run_bass_kernel_spmd(
        nc, in_maps, list(range(2 * nb)), trace=trace, trace_cores=[0]
    )
    outs = [np.asarray(r["out"]) for r in res.results]
    full = np.stack([outs[2 * b] + outs[2 * b + 1] for b in range(nb)])

    query_mask = np.asarray(inputs["query_mask"])
    if not query_mask.all():
        # masked query rows: reference yields uniform attention over all kv
        for b in range(nb):
            rows = ~query_mask[b]
            if rows.any():
                V = key_value[b] @ w_v  # [tkv, 256]
                meanV = V.mean(axis=0)  # [256]
                group = N_HEADS // NUM_KV_HEADS
                feat = np.concatenate(
                    [
                        meanV.reshape(NUM_KV_HEADS, D_K)[h // group]
                        for h in range(N_HEADS)
                    ]
                )
                full[b, rows, :] = feat @ w_out
    return full.astype(np.float32), res


def kernel(**inputs):
    out, _ = _run(inputs, trace=False)
    return out


def kernel_traced(**inputs):
    out, res = _run(inputs, trace=True)
    return out, res


if __name__ == "__main__":
    print("kernel.py is a library; use test.py")


# revision 15
# speedup vs baseline: 2.0768x; 1.2032x over previous
"""Cross-attention (GQA + RoPE) Trainium2 Bass kernel.

Sharding: 8 cores = 4 batches x 2 head-groups.
  core i -> batch b = i // 2, head-group g = i % 2
  Each core computes 8 query heads / 2 kv heads of one batch and a
  row-parallel partial of the output projection; the host sums the two
  partials per batch.

Key optimizations over the v1 baseline (591us):
  * kv compaction: ~50% of kv positions are masked out (kv_mask); the host
    gathers only the valid positions (plus padding to a multiple of 128),
    which is mathematically exact for softmax and nearly halves the
    attention-phase work on every engine.
  * bf16 operands end-to-end (weights, activations, probs): halves DMA and
    SBUF footprint, avoids the f32r narrow-matmul penalty; PSUM accumulation
    stays f32.  Measured end-to-end rel err ~6e-3 (gate 2e-2).
  * softmax denominators inverted with reciprocal_approx_fast (single custom
    DVE op, ~1.2us vs 6.5us for the exact reciprocal).  The custom op only
    works on base-partition-0 inputs, so the denominator row is first moved
    from PSUM partition 64 with an ACT Copy (Copy lives in every ACT table,
    so no table-load thrash against Exp).  The inverse is partition-broadcast
    on GpSimd, off the busy engines.
  * software pipelining: the attention inner loop (scores -> exp -> PV) is
    ACT-bound per chunk, which used to stall the PE every chunk and drop it
    to the mid p-state clock (2x slower matmuls).  Now the Q projection of
    the NEXT head pair and the output projection of the PREVIOUS tq block are
    emitted as filler matmuls inside the attention chunk loop, keeping the PE
    continuously busy at the peak p-state.

Per-core layout (all "T" tensors have head_dim / feature on partitions):
  qT   [1024, TQ]    query^T             (host-transposed, bf16)
  kvT  [1024, TKVC]  compacted key_value^T (bf16)
  wq   [1024, 512]   w_q columns of this head group, head-PERMUTED so that
                     pair-tile j holds local heads (j, j+4) -> rows (0-63,
                     64-127), making Q/K partition bases match per head.
  wk   [1024, 128]   w_k columns (2 kv heads)         (bf16)
  wv   [1024, 128]   w_v columns                      (bf16)
  wout [512, 1024]   w_out rows, same head permutation (bf16)
  kcos/ksin [128, TKVC] rope tables gathered at the kept kv positions (f32)
  qcos/qsin [128, TQ]   rope tables for query positions 0..TQ-1 (f32)
  maskb [128, NCH]   additive bias per 128-chunk: 0 valid / -30000 padding
  onesc [128, 64]    ones (bf16) for the V ones-column

PSUM map (exactly 8 banks):
  sc   2 x [128, t2] f32 (4 banks) - scores ring, consumed by exp
  fill 2 x [128, 512] f32 (2 banks) - K/V/Q projections + out projection
  acc  1 x [65, t2]  f32 (2 banks) - PV accumulator (row 64 = denominator)
"""

import os
from collections import deque
from contextlib import ExitStack

import numpy as np

import concourse.bass as bass
import concourse.bacc as bacc
import concourse.mybir as mybir
import concourse.tile as tile
from concourse.bass_utils import run_bass_kernel_spmd

F32 = mybir.dt.float32
BF16 = mybir.dt.bfloat16

D_MODEL = 1024
N_HEADS = 16
NUM_KV_HEADS = 4
D_K = 64
ROPE_BASE = 10000.0
B = 4
TQ = 2048
N_CORES = 8

NEG_BIAS = -30000.0


def _widths(total, w=512):
    out = [w] * (total // w)
    if total % w:
        out.append(total % w)
    return out


def build_bass(tq=TQ, tkv=1152, t2=1024):
    """Build the single-core SPMD program (same program on all 8 cores)."""
    nc = bacc.Bacc("TRN2", target_bir_lowering=False, debug=False)
    P = 128
    NCH = tkv // 128          # attention kv chunks
    NT2 = tq // t2            # tq blocks
    NHALF = t2 // 512         # 512-wide slices per tq block
    NPAIR = 4                 # head-pair tiles per core
    DT = BF16

    qT = nc.dram_tensor("qT", [D_MODEL, tq], DT, kind="ExternalInput").ap()
    kvT = nc.dram_tensor("kvT", [D_MODEL, tkv], DT, kind="ExternalInput").ap()
    wq = nc.dram_tensor("wq", [D_MODEL, 512], DT, kind="ExternalInput").ap()
    wk = nc.dram_tensor("wk", [D_MODEL, 128], DT, kind="ExternalInput").ap()
    wv = nc.dram_tensor("wv", [D_MODEL, 128], DT, kind="ExternalInput").ap()
    wout = nc.dram_tensor("wout", [512, D_MODEL], DT, kind="ExternalInput").ap()
    kcos = nc.dram_tensor("kcos", [P, tkv], F32, kind="ExternalInput").ap()
    ksin = nc.dram_tensor("ksin", [P, tkv], F32, kind="ExternalInput").ap()
    qcos = nc.dram_tensor("qcos", [P, tq], F32, kind="ExternalInput").ap()
    qsin = nc.dram_tensor("qsin", [P, tq], F32, kind="ExternalInput").ap()
    maskb = nc.dram_tensor("maskb", [P, NCH], F32, kind="ExternalInput").ap()
    onesc = nc.dram_tensor("onesc", [P, 64], DT, kind="ExternalInput").ap()
    out = nc.dram_tensor("out", [tq, D_MODEL], F32, kind="ExternalOutput").ap()

    with tile.TileContext(nc) as tc, ExitStack() as ctx:
        const = ctx.enter_context(tc.tile_pool(name="const", bufs=1))
        qpool = ctx.enter_context(tc.tile_pool(name="qpool", bufs=2))
        apool = ctx.enter_context(tc.tile_pool(name="apool", bufs=2))
        workp = ctx.enter_context(tc.tile_pool(name="workp", bufs=2))
        ropep = ctx.enter_context(tc.tile_pool(name="ropep", bufs=2))
        outp = ctx.enter_context(tc.tile_pool(name="outp", bufs=2))
        pp_sc = ctx.enter_context(tc.tile_pool(name="pp_sc", bufs=2, space="PSUM"))
        pp_fill = ctx.enter_context(tc.tile_pool(name="pp_fill", bufs=2, space="PSUM"))
        pp_acc = ctx.enter_context(tc.tile_pool(name="pp_acc", bufs=1, space="PSUM"))

        def MM(out_ap, lhsT, rhs, start, stop):
            return nc.tensor.matmul(out_ap, lhsT, rhs, start=start, stop=stop)

        # ---- constants / weights -------------------------------------------------
        # DMA issue is split across the GpSimd and Sync trigger queues, and
        # the big resident tensors are loaded in column pieces so dependent
        # matmuls start as soon as their slice lands (subtile deps).
        wk_sb = const.tile([P, 8, 128], DT)
        nc.gpsimd.dma_start(out=wk_sb, in_=wk.rearrange("(c p) f -> p c f", p=P))
        kvT_sb = const.tile([P, 8, tkv], DT)
        kvT_r = kvT.rearrange("(c p) t -> p c t", p=P)
        c0 = 0
        for w in _widths(tkv):
            nc.gpsimd.dma_start(
                out=kvT_sb[:, :, c0 : c0 + w], in_=kvT_r[:, :, c0 : c0 + w]
            )
            c0 += w
        wq_sb = const.tile([P, 8, 512], DT)
        nc.gpsimd.dma_start(out=wq_sb, in_=wq.rearrange("(c p) f -> p c f", p=P))
        qT_sb = const.tile([P, 8, tq], DT)
        qT_r = qT.rearrange("(c p) t -> p c t", p=P)
        for c0 in range(0, tq, 512):
            nc.gpsimd.dma_start(
                out=qT_sb[:, :, c0 : c0 + 512], in_=qT_r[:, :, c0 : c0 + 512]
            )
        wv_sb = const.tile([P, 8, 128], DT)
        nc.sync.dma_start(out=wv_sb, in_=wv.rearrange("(c p) f -> p c f", p=P))
        kcos_sb = const.tile([P, tkv], F32)
        nc.sync.dma_start(out=kcos_sb, in_=kcos)
        ksin_sb = const.tile([P, tkv], F32)
        nc.sync.dma_start(out=ksin_sb, in_=ksin)
        qcos_sb = const.tile([P, tq], F32)
        nc.sync.dma_start(out=qcos_sb, in_=qcos)
        qsin_sb = const.tile([P, tq], F32)
        nc.sync.dma_start(out=qsin_sb, in_=qsin)
        wout_sb = const.tile([P, 4, D_MODEL], DT)
        nc.sync.dma_start(out=wout_sb, in_=wout.rearrange("(c p) f -> p c f", p=P))
        mask_sb = const.tile([P, NCH], F32)
        nc.sync.dma_start(out=mask_sb, in_=maskb)

        Kt = const.tile([P, tkv], DT)
        Vt = [const.tile([P, NCH * 65], DT, name=f"Vt{i}") for i in range(2)]
        for i in range(2):
            nc.sync.dma_start(
                out=Vt[i].rearrange("p (c k) -> p c k", k=65)[:, :, 64],
                in_=onesc[:, :NCH],
            )

        def rope_apply(dest, ps, cos_sb, sin_sb, col0, width):
            """dest[128, width] (SBUF) = rope(ps[128, width] PSUM), table
            cols col0..col0+width. Rows: two stacked heads, each [x1;x2]."""
            cs = cos_sb[:, col0 : col0 + width]
            t_cos = ropep.tile([P, 512], F32, tag="rope", name="t_cos")
            t_u = ropep.tile([P, 512], F32, tag="rope", name="t_u")
            tc_ = t_cos[:, :width]
            tu_ = t_u[:, :width]
            nc.vector.tensor_mul(tc_, ps, cs)
            for b0 in (0, 64):
                # sin rows [b0:b0+32] = -sin, [b0+32:b0+64] = +sin
                nc.vector.tensor_mul(
                    tu_[b0 : b0 + 32, :],
                    ps[b0 + 32 : b0 + 64, :],
                    sin_sb[b0 : b0 + 32, col0 : col0 + width],
                )
                nc.vector.tensor_mul(
                    tu_[b0 + 32 : b0 + 64, :],
                    ps[b0 : b0 + 32, :],
                    sin_sb[b0 + 32 : b0 + 64, col0 : col0 + width],
                )
            nc.vector.tensor_add(dest, tc_, tu_)

        # ---- phase KV: K/V projections ------------------------------------------
        col0 = 0
        for w in _widths(tkv):
            ps_k = pp_fill.tile([P, 512], F32, tag="fill", name="ps_k")
            pk = ps_k[:, :w]
            for d in range(8):
                MM(pk, wk_sb[:, d, :], kvT_sb[:, d, col0 : col0 + w], d == 0, d == 7)
            rope_apply(Kt[:, col0 : col0 + w], pk, kcos_sb, ksin_sb, col0, w)
            for s in range(w // 128):
                ps_v = pp_fill.tile([P, 512], F32, tag="fill", name="ps_v")
                pv = ps_v[:, 0:128]
                k0 = col0 + s * 128
                for d in range(8):
                    MM(pv, kvT_sb[:, d, k0 : k0 + 128], wv_sb[:, d, :], d == 0, d == 7)
                c = k0 // 128
                # ACT is idle during the KV phase; keep the DVE free for rope
                nc.scalar.copy(out=Vt[0][:, c * 65 : c * 65 + 64], in_=pv[:, 0:64])
                nc.scalar.copy(out=Vt[1][:, c * 65 : c * 65 + 64], in_=pv[:, 64:128])
            col0 += w

        # ---- filler machinery ----------------------------------------------------
        fillers = deque()

        def pump(n=1):
            for _ in range(n):
                if fillers:
                    fillers.popleft()()

        def drain():
            while fillers:
                fillers.popleft()()

        Qts = {}      # (j, it2) -> Qt tile
        attnTs = {}   # (j, it2) -> attnT tile

        def push_qproj(j, it2):
            """Emit Q projection + rope for pair j / block it2 as 4 fillers."""
            qt = qpool.tile([P, t2], DT, tag=f"Q{j}", name=f"Qt{j}")
            Qts[(j, it2)] = qt

            def half_closure(half):
                def f():
                    ps = pp_fill.tile([P, 512], F32, tag="fill", name="ps_qh")
                    c0 = it2 * t2 + half * 512
                    for d in range(8):
                        MM(
                            ps,
                            wq_sb[:, d, j * 128 : (j + 1) * 128],
                            qT_sb[:, d, c0 : c0 + 512],
                            d == 0,
                            d == 7,
                        )
                    rope_apply(
                        qt[:, half * 512 : (half + 1) * 512],
                        ps,
                        qcos_sb,
                        qsin_sb,
                        it2 * t2 + half * 512,
                        512,
                    )

                return f

            for half in range(NHALF):
                fillers.append(half_closure(half))

        def push_outproj(it2):
            """Emit output projection of block it2 as per-s-chunk fillers."""
            ats = [attnTs[(j, it2)] for j in range(NPAIR)]

            def s_closure(s, n):
                def f():
                    ps_f = pp_fill.tile([P, 512], F32, tag="fill", name="ps_f")
                    for p_ in range(NPAIR):
                        MM(
                            ps_f,
                            ats[p_][:, s * 128 : (s + 1) * 128],
                            wout_sb[:, p_, n * 512 : (n + 1) * 512],
                            p_ == 0,
                            p_ == NPAIR - 1,
                        )
                    ob = obs[s]
                    nc.vector.tensor_copy(
                        out=ob[:, n * 512 : (n + 1) * 512], in_=ps_f
                    )
                    if n == 1:
                        r0 = it2 * t2 + s * 128
                        nc.sync.dma_start(out=out[r0 : r0 + 128, :], in_=ob)

                return f

            obs = {}
            for s in range(t2 // 128):
                obs[s] = outp.tile([P, D_MODEL], F32, tag="ob", name="ob", bufs=3)
                for n in range(2):
                    fillers.append(s_closure(s, n))

        def push_norm(U, den, j, it2, base):
            """Deferred normalization: reciprocal -> broadcast -> scale."""
            inv = workp.tile([1, t2], F32, tag="inv", name="inv", bufs=2)
            invb = workp.tile([64, t2], F32, tag="invb", name="invb", bufs=2)

            def f1():
                nc.vector.reciprocal_approx_fast(out=inv, in_=den)
                nc.gpsimd.partition_broadcast(invb, inv)

            def f2():
                nc.vector.tensor_mul(
                    attnTs[(j, it2)][base : base + 64, :], U[0:64, :], invb
                )

            fillers.append(f1)
            fillers.append(f2)

        # ---- attention -----------------------------------------------------------
        push_qproj(0, 0)
        drain()

        for it2 in range(NT2):
            for j in range(NPAIR):
                attnTs[(j, it2)] = apool.tile(
                    [P, t2], DT, tag=f"A{j}", name=f"attnT{j}"
                )
                # queue next pair's Q projection as filler work
                if j + 1 < NPAIR:
                    push_qproj(j + 1, it2)
                elif it2 + 1 < NT2:
                    push_qproj(0, it2 + 1)
                # queue previous block's output projection (pairs 1..3)
                if it2 > 0 and j == 1:
                    push_outproj(it2 - 1)

                qt = Qts[(j, it2)]
                for ab, (kvh, base) in enumerate([(0, 0), (1, 64)]):
                    ps_o = pp_acc.tile([65, t2], F32, tag="acc", name="ps_o")
                    prev = None
                    for c in range(NCH):
                        ps_s = pp_sc.tile([P, t2], F32, tag="sc", name="ps_s")
                        for half in range(NHALF):
                            MM(
                                ps_s[:, half * 512 : (half + 1) * 512],
                                Kt[base : base + 64, c * 128 : (c + 1) * 128],
                                qt[base : base + 64, half * 512 : (half + 1) * 512],
                                True,
                                True,
                            )
                        ex = workp.tile([P, t2], DT, tag="expT", name="ex", bufs=4)
                        nc.scalar.activation(
                            out=ex,
                            in_=ps_s,
                            func=mybir.ActivationFunctionType.Exp,
                            bias=mask_sb[:, c : c + 1],
                            scale=0.125,
                        )
                        pump(1)
                        if prev is not None:
                            for half in range(NHALF):
                                MM(
                                    ps_o[:, half * 512 : (half + 1) * 512],
                                    Vt[kvh][:, (c - 1) * 65 : (c - 1) * 65 + 65],
                                    prev[:, half * 512 : (half + 1) * 512],
                                    c - 1 == 0,
                                    c - 1 == NCH - 1,
                                )
                        else:
                            pump(1)
                        prev = ex
                    for half in range(NHALF):
                        MM(
                            ps_o[:, half * 512 : (half + 1) * 512],
                            Vt[kvh][:, (NCH - 1) * 65 : (NCH - 1) * 65 + 65],
                            prev[:, half * 512 : (half + 1) * 512],
                            NCH - 1 == 0,
                            True,
                        )
                    # free the accumulator promptly: U copy on DVE, the
                    # denominator row to partition 0 via ACT Copy (no table
                    # load; reciprocal_approx_fast needs base partition 0).
                    U = workp.tile([65, t2], F32, tag="unorm", name="U", bufs=3)
                    nc.vector.tensor_copy(out=U, in_=ps_o)
                    den = workp.tile([1, t2], F32, tag="den", name="den", bufs=2)
                    nc.scalar.copy(out=den, in_=ps_o[64:65, :])
                    push_norm(U, den, j, it2, base)
            drain()

        # ---- tail: output projection of the last block ---------------------------
        push_outproj(NT2 - 1)
        drain()

    nc.compile()
    return nc


# ---------------------------------------------------------------------------
# host-side sharding / prep
# ---------------------------------------------------------------------------

_HEAD_PERM = [0, 4, 1, 5, 2, 6, 3, 7]  # local head order inside pair tiles

try:
    import ml_dtypes

    _BF16 = ml_dtypes.bfloat16
except ImportError:  # pragma: no cover
    import jax.numpy as jnp

    _BF16 = jnp.bfloat16


def _bf(x):
    return np.ascontiguousarray(np.asarray(x, dtype=np.float32).astype(_BF16))


def _rope_tables(positions):
    """cos/sin tables [128, len(positions)] stacked for two heads."""
    theta = ROPE_BASE ** (-np.arange(0, D_K, 2, dtype=np.float32) / D_K)  # [32]
    pos = np.asarray(positions, dtype=np.float32)[:, None]  # [T,1]
    ang = pos * theta[None, :]  # [T,32]
    c = np.cos(ang).T.astype(np.float32)  # [32, T]
    s = np.sin(ang).T.astype(np.float32)
    cosF = np.concatenate([c, c, c, c], axis=0)
    sinF = np.concatenate([-s, s, -s, s], axis=0)
    return np.ascontiguousarray(cosF), np.ascontiguousarray(sinF)


def make_in_maps(query, key_value, kv_mask, w_q, w_k, w_v, w_out, tq, tkv_c):
    nb = query.shape[0]
    qcos, qsin = _rope_tables(np.arange(tq))
    in_maps = []
    col_perm = np.concatenate(
        [np.arange(h * D_K, (h + 1) * D_K) for h in _HEAD_PERM]
    )
    for core in range(2 * nb):
        b = core // 2
        g = core % 2
        idx = np.nonzero(kv_mask[b])[0]
        nv = len(idx)
        kv_c = np.zeros((tkv_c, key_value.shape[2]), np.float32)
        kv_c[:nv] = key_value[b][idx]
        pos = np.zeros(tkv_c, np.int64)
        pos[:nv] = idx
        kcos, ksin = _rope_tables(pos)
        maskb = np.full(tkv_c, NEG_BIAS, np.float32)
        maskb[:nv] = 0.0
        maskb = np.ascontiguousarray(maskb.reshape(tkv_c // 128, 128).T)

        qTb = np.ascontiguousarray(_bf(query[b]).T)
        kvTb = np.ascontiguousarray(_bf(kv_c).T)
        wq_g = w_q[:, g * 512 : (g + 1) * 512][:, col_perm]
        wk_g = w_k[:, g * 128 : (g + 1) * 128]
        wv_g = w_v[:, g * 128 : (g + 1) * 128]
        wout_g = w_out[g * 512 : (g + 1) * 512, :][col_perm, :]
        in_maps.append(
            {
                "qT": qTb,
                "kvT": kvTb,
                "wq": _bf(wq_g),
                "wk": _bf(wk_g),
                "wv": _bf(wv_g),
                "wout": _bf(wout_g),
                "kcos": kcos,
                "ksin": ksin,
                "qcos": qcos,
                "qsin": qsin,
                "maskb": maskb,
                "onesc": _bf(np.ones((128, 64), np.float32)),
            }
        )
    return in_maps


_NC_CACHE = {}


def _get_nc(tq, tkv_c, t2=1024):
    key = (tq, tkv_c, t2)
    if key not in _NC_CACHE:
        _NC_CACHE[key] = build_bass(tq, tkv_c, t2)
    return _NC_CACHE[key]


def _run(inputs, trace=False):
    query = np.asarray(inputs["query"], dtype=np.float32)
    key_value = np.asarray(inputs["key_value"], dtype=np.float32)
    kv_mask = np.asarray(inputs["kv_mask"])
    w_q = np.asarray(inputs["w_q"], dtype=np.float32)
    w_k = np.asarray(inputs["w_k"], dtype=np.float32)
    w_v = np.asarray(inputs["w_v"], dtype=np.float32)
    w_out = np.asarray(inputs["w_out"], dtype=np.float32)
    nb, tq, _ = query.shape

    max_valid = int(kv_mask.sum(axis=1).max())
    tkv_c = max(256, -(-max_valid // 128) * 128)

    nc = _get_nc(tq, tkv_c)
    in_maps = make_in_maps(
        query, key_value, kv_mask, w_q, w_k, w_v, w_out, tq, tkv_c
    )
    res = run_bass_kernel_spmd(
        nc, in_maps, list(range(2 * nb)), trace=trace, trace_cores=[0]
    )
    outs = [np.asarray(r["out"]) for r in res.results]
    full = np.stack([outs[2 * b] + outs[2 * b + 1] for b in range(nb)])

    query_mask = np.asarray(inputs["query_mask"])
    if not query_mask.all():
        # masked query rows: reference yields uniform attention over all kv
        for b in range(nb):
            rows = ~query_mask[b]
            if rows.any():
                V = key_value[b] @ w_v  # [tkv, 256]
                meanV = V.mean(axis=0)  # [256]
                group = N_HEADS // NUM_KV_HEADS
                feat = np.concatenate(
                    [
                        meanV.reshape(NUM_KV_HEADS, D_K)[h // group]
                        for h in range(N_HEADS)
                    ]
                )
                full[b, rows, :] = feat @ w_out
    return full.astype(np.float32), res


def kernel(**inputs):
    out, _ = _run(inputs, trace=False)
    return out


def kernel_traced(**inputs):
    out, res = _run(inputs, trace=True)
    return out, res


if __name__ == "__main__":
    print("kernel.py is a library; use test.py")


# revision 19
# speedup vs baseline: 2.1758x; 1.0477x over previous
"""Cross-attention (GQA + RoPE) Trainium2 Bass kernel.

Sharding: 8 cores = 4 batches x 2 head-groups.
  core i -> batch b = i // 2, head-group g = i % 2
  Each core computes 8 query heads / 2 kv heads of one batch and a
  row-parallel partial of the output projection; the host sums the two
  partials per batch.

Key optimizations over the v1 baseline (591us):
  * kv compaction: ~50% of kv positions are masked out (kv_mask); the host
    gathers only the valid positions (plus padding to a multiple of 128),
    which is mathematically exact for softmax and nearly halves the
    attention-phase work on every engine.
  * bf16 operands end-to-end (weights, activations, probs): halves DMA and
    SBUF footprint, avoids the f32r narrow-matmul penalty; PSUM accumulation
    stays f32.  Measured end-to-end rel err ~6e-3 (gate 2e-2).
  * softmax denominators inverted with reciprocal_approx_fast (single custom
    DVE op, ~1.2us vs 6.5us for the exact reciprocal).  The custom op only
    works on base-partition-0 inputs, so the denominator row is first moved
    from PSUM partition 64 with an ACT Copy (Copy lives in every ACT table,
    so no table-load thrash against Exp).  The inverse is partition-broadcast
    on GpSimd, off the busy engines.
  * software pipelining: the attention inner loop (scores -> exp -> PV) is
    ACT-bound per chunk, which used to stall the PE every chunk and drop it
    to the mid p-state clock (2x slower matmuls).  Now the Q projection of
    the NEXT head pair and the output projection of the PREVIOUS tq block are
    emitted as filler matmuls inside the attention chunk loop, keeping the PE
    continuously busy at the peak p-state.

Per-core layout (all "T" tensors have head_dim / feature on partitions):
  qT   [1024, TQ]    query^T             (host-transposed, bf16)
  kvT  [1024, TKVC]  compacted key_value^T (bf16)
  wq   [1024, 512]   w_q columns of this head group, head-PERMUTED so that
                     pair-tile j holds local heads (j, j+4) -> rows (0-63,
                     64-127), making Q/K partition bases match per head.
  wk   [1024, 128]   w_k columns (2 kv heads)         (bf16)
  wv   [1024, 128]   w_v columns                      (bf16)
  wout [512, 1024]   w_out rows, same head permutation (bf16)
  kcos/ksin [128, TKVC] rope tables gathered at the kept kv positions (f32)
  qcos/qsin [128, TQ]   rope tables for query positions 0..TQ-1 (f32)
  maskb [128, NCH]   additive bias per 128-chunk: 0 valid / -30000 padding
  onesc [128, 64]    ones (bf16) for the V ones-column

PSUM map (exactly 8 banks):
  sc   2 x [128, t2] f32 (4 banks) - scores ring, consumed by exp
  fill 2 x [128, 512] f32 (2 banks) - K/V/Q projections + out projection
  acc  1 x [65, t2]  f32 (2 banks) - PV accumulator (row 64 = denominator)
"""

import os
from collections import deque
from contextlib import ExitStack

import numpy as np

import concourse.bass as bass
import concourse.bacc as bacc
import concourse.mybir as mybir
import concourse.tile as tile
from concourse.bass_utils import run_bass_kernel_spmd

F32 = mybir.dt.float32
BF16 = mybir.dt.bfloat16

D_MODEL = 1024
N_HEADS = 16
NUM_KV_HEADS = 4
D_K = 64
ROPE_BASE = 10000.0
B = 4
TQ = 2048
N_CORES = 8

NEG_BIAS = -30000.0


def _widths(total, w=512):
    out = [w] * (total // w)
    if total % w:
        out.append(total % w)
    return out


def build_bass(tq=TQ, tkv=1152, t2=1024):
    """Build the single-core SPMD program (same program on all 8 cores)."""
    nc = bacc.Bacc("TRN2", target_bir_lowering=False, debug=False)
    P = 128
    NCH = tkv // 128          # attention kv chunks
    NT2 = tq // t2            # tq blocks
    NHALF = t2 // 512         # 512-wide slices per tq block
    NPAIR = 4                 # head-pair tiles per core
    DT = BF16

    qT = nc.dram_tensor("qT", [D_MODEL, tq], DT, kind="ExternalInput").ap()
    kvT = nc.dram_tensor("kvT", [D_MODEL, tkv], DT, kind="ExternalInput").ap()
    wq = nc.dram_tensor("wq", [D_MODEL, 512], DT, kind="ExternalInput").ap()
    wk = nc.dram_tensor("wk", [D_MODEL, 128], DT, kind="ExternalInput").ap()
    wv = nc.dram_tensor("wv", [D_MODEL, 128], DT, kind="ExternalInput").ap()
    wout = nc.dram_tensor("wout", [512, D_MODEL], DT, kind="ExternalInput").ap()
    kcos = nc.dram_tensor("kcos", [P, tkv], F32, kind="ExternalInput").ap()
    ksin = nc.dram_tensor("ksin", [P, tkv], F32, kind="ExternalInput").ap()
    qcos = nc.dram_tensor("qcos", [P, tq], F32, kind="ExternalInput").ap()
    qsin = nc.dram_tensor("qsin", [P, tq], F32, kind="ExternalInput").ap()
    maskb = nc.dram_tensor("maskb", [P, NCH], F32, kind="ExternalInput").ap()
    onesc = nc.dram_tensor("onesc", [P, 64], DT, kind="ExternalInput").ap()
    out = nc.dram_tensor("out", [tq, D_MODEL], F32, kind="ExternalOutput").ap()

    with tile.TileContext(nc) as tc, ExitStack() as ctx:
        const = ctx.enter_context(tc.tile_pool(name="const", bufs=1))
        qpool = ctx.enter_context(tc.tile_pool(name="qpool", bufs=2))
        apool = ctx.enter_context(tc.tile_pool(name="apool", bufs=2))
        workp = ctx.enter_context(tc.tile_pool(name="workp", bufs=2))
        ropep = ctx.enter_context(tc.tile_pool(name="ropep", bufs=2))
        outp = ctx.enter_context(tc.tile_pool(name="outp", bufs=2))
        pp_sc = ctx.enter_context(tc.tile_pool(name="pp_sc", bufs=2, space="PSUM"))
        pp_fill = ctx.enter_context(tc.tile_pool(name="pp_fill", bufs=2, space="PSUM"))
        pp_acc = ctx.enter_context(tc.tile_pool(name="pp_acc", bufs=1, space="PSUM"))

        def MM(out_ap, lhsT, rhs, start, stop):
            return nc.tensor.matmul(out_ap, lhsT, rhs, start=start, stop=stop)

        # ---- constants / weights -------------------------------------------------
        # DMA issue is split across the GpSimd and Sync trigger queues, and
        # the big resident tensors are loaded in column pieces so dependent
        # matmuls start as soon as their slice lands (subtile deps).
        wk_sb = const.tile([P, 8, 128], DT)
        nc.gpsimd.dma_start(out=wk_sb, in_=wk.rearrange("(c p) f -> p c f", p=P))
        kvT_sb = const.tile([P, 8, tkv], DT)
        kvT_r = kvT.rearrange("(c p) t -> p c t", p=P)
        c0 = 0
        for w in _widths(tkv):
            nc.gpsimd.dma_start(
                out=kvT_sb[:, :, c0 : c0 + w], in_=kvT_r[:, :, c0 : c0 + w]
            )
            c0 += w
        wq_sb = const.tile([P, 8, 512], DT)
        nc.gpsimd.dma_start(out=wq_sb, in_=wq.rearrange("(c p) f -> p c f", p=P))
        qT_sb = const.tile([P, 8, tq], DT)
        qT_r = qT.rearrange("(c p) t -> p c t", p=P)
        for c0 in range(0, tq, 512):
            nc.gpsimd.dma_start(
                out=qT_sb[:, :, c0 : c0 + 512], in_=qT_r[:, :, c0 : c0 + 512]
            )
        wv_sb = const.tile([P, 8, 128], DT)
        nc.sync.dma_start(out=wv_sb, in_=wv.rearrange("(c p) f -> p c f", p=P))
        kcos_sb = const.tile([P, tkv], F32)
        nc.sync.dma_start(out=kcos_sb, in_=kcos)
        ksin_sb = const.tile([P, tkv], F32)
        nc.sync.dma_start(out=ksin_sb, in_=ksin)
        qcos_sb = const.tile([P, tq], F32)
        nc.sync.dma_start(out=qcos_sb, in_=qcos)
        qsin_sb = const.tile([P, tq], F32)
        nc.sync.dma_start(out=qsin_sb, in_=qsin)
        wout_sb = const.tile([P, 4, D_MODEL], DT)
        nc.sync.dma_start(out=wout_sb, in_=wout.rearrange("(c p) f -> p c f", p=P))
        mask_sb = const.tile([P, NCH], F32)
        nc.sync.dma_start(out=mask_sb, in_=maskb)

        Kt = const.tile([P, tkv], DT)
        # V tiles hold, per kv chunk, [ones | 0 x 63 | V(64)]: the PV matmul
        # then writes the softmax denominator to PSUM partition 0 (where the
        # custom-DVE reciprocal can read it directly) and the weighted values
        # to partitions 64..127 -- all partition-aligned, no extra copies.
        # The matmul cost is unchanged (n-bound, independent of m).
        Vt = [const.tile([P, NCH * 128], DT, name=f"Vt{i}") for i in range(2)]
        for i in range(2):
            nc.vector.memset(
                Vt[i].rearrange("p (c k) -> p c k", k=128)[:, :, 1:64], 0.0
            )
            nc.sync.dma_start(
                out=Vt[i].rearrange("p (c k) -> p c k", k=128)[:, :, 0],
                in_=onesc[:, :NCH],
            )

        def rope_apply(dest, ps, cos_sb, sin_sb, col0, width):
            """dest[128, width] (SBUF) = rope(ps[128, width] PSUM), table
            cols col0..col0+width. Rows: two stacked heads, each [x1;x2]."""
            cs = cos_sb[:, col0 : col0 + width]
            t_cos = ropep.tile([P, 512], F32, tag="rope", name="t_cos")
            t_u = ropep.tile([P, 512], F32, tag="rope", name="t_u")
            tc_ = t_cos[:, :width]
            tu_ = t_u[:, :width]
            nc.vector.tensor_mul(tc_, ps, cs)
            for b0 in (0, 64):
                # sin rows [b0:b0+32] = -sin, [b0+32:b0+64] = +sin
                nc.vector.tensor_mul(
                    tu_[b0 : b0 + 32, :],
                    ps[b0 + 32 : b0 + 64, :],
                    sin_sb[b0 : b0 + 32, col0 : col0 + width],
                )
                nc.vector.tensor_mul(
                    tu_[b0 + 32 : b0 + 64, :],
                    ps[b0 : b0 + 32, :],
                    sin_sb[b0 + 32 : b0 + 64, col0 : col0 + width],
                )
            nc.vector.tensor_add(dest, tc_, tu_)

        # ---- phase KV: K/V projections ------------------------------------------
        col0 = 0
        for w in _widths(tkv):
            ps_k = pp_fill.tile([P, 512], F32, tag="fill", name="ps_k")
            pk = ps_k[:, :w]
            for d in range(8):
                MM(pk, wk_sb[:, d, :], kvT_sb[:, d, col0 : col0 + w], d == 0, d == 7)
            rope_apply(Kt[:, col0 : col0 + w], pk, kcos_sb, ksin_sb, col0, w)
            for s in range(w // 128):
                ps_v = pp_fill.tile([P, 512], F32, tag="fill", name="ps_v")
                pv = ps_v[:, 0:128]
                k0 = col0 + s * 128
                for d in range(8):
                    MM(pv, kvT_sb[:, d, k0 : k0 + 128], wv_sb[:, d, :], d == 0, d == 7)
                c = k0 // 128
                # ACT is idle during the KV phase; keep the DVE free for rope
                nc.scalar.copy(
                    out=Vt[0][:, c * 128 + 64 : c * 128 + 128], in_=pv[:, 0:64]
                )
                nc.scalar.copy(
                    out=Vt[1][:, c * 128 + 64 : c * 128 + 128], in_=pv[:, 64:128]
                )
            col0 += w

        # ---- filler machinery ----------------------------------------------------
        fillers = deque()

        def pump(n=1):
            for _ in range(n):
                if fillers:
                    fillers.popleft()()

        def drain():
            while fillers:
                fillers.popleft()()

        Qts = {}      # (j, it2) -> Qt tile
        attnTs = {}   # (j, it2) -> attnT tile

        def push_qproj(j, it2):
            """Emit Q projection + rope for pair j / block it2 as 4 fillers."""
            qt = qpool.tile([P, t2], DT, tag=f"Q{j}", name=f"Qt{j}")
            Qts[(j, it2)] = qt

            def half_closure(half):
                def f():
                    ps = pp_fill.tile([P, 512], F32, tag="fill", name="ps_qh")
                    c0 = it2 * t2 + half * 512
                    for d in range(8):
                        MM(
                            ps,
                            wq_sb[:, d, j * 128 : (j + 1) * 128],
                            qT_sb[:, d, c0 : c0 + 512],
                            d == 0,
                            d == 7,
                        )
                    rope_apply(
                        qt[:, half * 512 : (half + 1) * 512],
                        ps,
                        qcos_sb,
                        qsin_sb,
                        it2 * t2 + half * 512,
                        512,
                    )

                return f

            for half in range(NHALF):
                fillers.append(half_closure(half))

        def push_outproj(it2):
            """Emit output projection of block it2 as per-s-chunk fillers."""
            ats = [attnTs[(j, it2)] for j in range(NPAIR)]

            def s_closure(s, n):
                def f():
                    ps_f = pp_fill.tile([P, 512], F32, tag="fill", name="ps_f")
                    for p_ in range(NPAIR):
                        MM(
                            ps_f,
                            ats[p_][:, s * 128 : (s + 1) * 128],
                            wout_sb[:, p_, n * 512 : (n + 1) * 512],
                            p_ == 0,
                            p_ == NPAIR - 1,
                        )
                    ob = obs[s]
                    nc.vector.tensor_copy(
                        out=ob[:, n * 512 : (n + 1) * 512], in_=ps_f
                    )
                    if n == 1:
                        r0 = it2 * t2 + s * 128
                        nc.sync.dma_start(out=out[r0 : r0 + 128, :], in_=ob)

                return f

            obs = {}
            for s in range(t2 // 128):
                obs[s] = outp.tile([P, D_MODEL], F32, tag="ob", name="ob", bufs=3)
                for n in range(2):
                    fillers.append(s_closure(s, n))

        def push_norm(U, den, j, it2, base):
            """Deferred normalization: reciprocal -> broadcast -> scale."""
            inv = workp.tile([1, t2], F32, tag="inv", name="inv", bufs=2)
            invb = workp.tile([64, t2], F32, tag="invb", name="invb", bufs=2)

            def f1():
                nc.vector.reciprocal_approx_fast(out=inv, in_=den)
                nc.gpsimd.partition_broadcast(invb, inv)

            def f2():
                nc.vector.tensor_mul(
                    attnTs[(j, it2)][base : base + 64, :], U, invb
                )

            fillers.append(f1)
            fillers.append(f2)

        # ---- attention -----------------------------------------------------------
        push_qproj(0, 0)
        drain()

        for it2 in range(NT2):
            for j in range(NPAIR):
                attnTs[(j, it2)] = apool.tile(
                    [P, t2], DT, tag=f"A{j}", name=f"attnT{j}"
                )
                # queue next pair's Q projection as filler work
                if j + 1 < NPAIR:
                    push_qproj(j + 1, it2)
                elif it2 + 1 < NT2:
                    push_qproj(0, it2 + 1)
                # queue previous block's output projection (pairs 1..3)
                if it2 > 0 and j == 1:
                    push_outproj(it2 - 1)

                qt = Qts[(j, it2)]
                for ab, (kvh, base) in enumerate([(0, 0), (1, 64)]):
                    ps_o = pp_acc.tile([P, t2], F32, tag="acc", name="ps_o")
                    prev = None
                    for c in range(NCH):
                        ps_s = pp_sc.tile([P, t2], F32, tag="sc", name="ps_s")
                        for half in range(NHALF):
                            MM(
                                ps_s[:, half * 512 : (half + 1) * 512],
                                Kt[base : base + 64, c * 128 : (c + 1) * 128],
                                qt[base : base + 64, half * 512 : (half + 1) * 512],
                                True,
                                True,
                            )
                        ex = workp.tile([P, t2], DT, tag="expT", name="ex", bufs=4)
                        nc.scalar.activation(
                            out=ex,
                            in_=ps_s,
                            func=mybir.ActivationFunctionType.Exp,
                            bias=mask_sb[:, c : c + 1],
                            scale=0.125,
                        )
                        pump(1)
                        if prev is not None:
                            for half in range(NHALF):
                                MM(
                                    ps_o[:, half * 512 : (half + 1) * 512],
                                    Vt[kvh][:, (c - 1) * 128 : c * 128],
                                    prev[:, half * 512 : (half + 1) * 512],
                                    c - 1 == 0,
                                    c - 1 == NCH - 1,
                                )
                        else:
                            pump(1)
                        prev = ex
                    for half in range(NHALF):
                        MM(
                            ps_o[:, half * 512 : (half + 1) * 512],
                            Vt[kvh][:, (NCH - 1) * 128 : NCH * 128],
                            prev[:, half * 512 : (half + 1) * 512],
                            NCH - 1 == 0,
                            True,
                        )
                    # free the accumulator promptly: U copy on DVE, the
                    # denominator row to partition 0 via ACT Copy (no table
                    # load; reciprocal_approx_fast needs base partition 0).
                    U = workp.tile([65, t2], F32, tag="unorm", name="U", bufs=3)
                    nc.vector.tensor_copy(out=U, in_=ps_o)
                    den = workp.tile([1, t2], F32, tag="den", name="den", bufs=2)
                    nc.scalar.copy(out=den, in_=ps_o[64:65, :])
                    push_norm(U, den, j, it2, base)
            drain()

        # ---- tail: output projection of the last block ---------------------------
        push_outproj(NT2 - 1)
        drain()

    nc.compile()
    return nc


# ---------------------------------------------------------------------------
# host-side sharding / prep
# ---------------------------------------------------------------------------

_HEAD_PERM = [0, 4, 1, 5, 2, 6, 3, 7]  # local head order inside pair tiles

try:
    import ml_dtypes

    _BF16 = ml_dtypes.bfloat16
except ImportError:  # pragma: no cover
    import jax.numpy as jnp

    _BF16 = jnp.bfloat16


def _bf(x):
    return np.ascontiguousarray(np.asarray(x, dtype=np.float32).astype(_BF16))


def _rope_tables(positions):
    """cos/sin tables [128, len(positions)] stacked for two heads."""
    theta = ROPE_BASE ** (-np.arange(0, D_K, 2, dtype=np.float32) / D_K)  # [32]
    pos = np.asarray(positions, dtype=np.float32)[:, None]  # [T,1]
    ang = pos * theta[None, :]  # [T,32]
    c = np.cos(ang).T.astype(np.float32)  # [32, T]
    s = np.sin(ang).T.astype(np.float32)
    cosF = np.concatenate([c, c, c, c], axis=0)
    sinF = np.concatenate([-s, s, -s, s], axis=0)
    return np.ascontiguousarray(cosF), np.ascontiguousarray(sinF)


def make_in_maps(query, key_value, kv_mask, w_q, w_k, w_v, w_out, tq, tkv_c):
    nb = query.shape[0]
    qcos, qsin = _rope_tables(np.arange(tq))
    in_maps = []
    col_perm = np.concatenate(
        [np.arange(h * D_K, (h + 1) * D_K) for h in _HEAD_PERM]
    )
    for core in range(2 * nb):
        b = core // 2
        g = core % 2
        idx = np.nonzero(kv_mask[b])[0]
        nv = len(idx)
        kv_c = np.zeros((tkv_c, key_value.shape[2]), np.float32)
        kv_c[:nv] = key_value[b][idx]
        pos = np.zeros(tkv_c, np.int64)
        pos[:nv] = idx
        kcos, ksin = _rope_tables(pos)
        maskb = np.full(tkv_c, NEG_BIAS, np.float32)
        maskb[:nv] = 0.0
        maskb = np.ascontiguousarray(maskb.reshape(tkv_c // 128, 128).T)

        qTb = np.ascontiguousarray(_bf(query[b]).T)
        kvTb = np.ascontiguousarray(_bf(kv_c).T)
        wq_g = w_q[:, g * 512 : (g + 1) * 512][:, col_perm]
        wk_g = w_k[:, g * 128 : (g + 1) * 128]
        wv_g = w_v[:, g * 128 : (g + 1) * 128]
        wout_g = w_out[g * 512 : (g + 1) * 512, :][col_perm, :]
        in_maps.append(
            {
                "qT": qTb,
                "kvT": kvTb,
                "wq": _bf(wq_g),
                "wk": _bf(wk_g),
                "wv": _bf(wv_g),
                "wout": _bf(wout_g),
                "kcos": _bf(kcos),
                "ksin": _bf(ksin),
                "qcos": _bf(qcos),
                "qsin": _bf(qsin),
                "maskb": maskb,
                "onesc": _bf(np.ones((128, 64), np.float32)),
            }
        )
    return in_maps


_NC_CACHE = {}


def _get_nc(tq, tkv_c, t2=1024):
    key = (tq, tkv_c, t2)
    if key not in _NC_CACHE:
        _NC_CACHE[key] = build_bass(tq, tkv_c, t2)
    return _NC_CACHE[key]


def _run(inputs, trace=False):
    query = np.asarray(inputs["query"], dtype=np.float32)
    key_value = np.asarray(inputs["key_value"], dtype=np.float32)
    kv_mask = np.asarray(inputs["kv_mask"])
    w_q = np.asarray(inputs["w_q"], dtype=np.float32)
    w_k = np.asarray(inputs["w_k"], dtype=np.float32)
    w_v = np.asarray(inputs["w_v"], dtype=np.float32)
    w_out = np.asarray(inputs["w_out"], dtype=np.float32)
    nb, tq, _ = query.shape

    max_valid = int(kv_mask.sum(axis=1).max())
    tkv_c = max(256, -(-max_valid // 128) * 128)

    nc = _get_nc(tq, tkv_c)
    in_maps = make_in_maps(
        query, key_value, kv_mask, w_q, w_k, w_v, w_out, tq, tkv_c
    )
    res = run_bass_kernel_spmd(
        nc, in_maps, list(range(2 * nb)), trace=trace, trace_cores=[0]
    )
    outs = [np.asarray(r["out"]).astype(np.float32) for r in res.results]
    full = np.stack([outs[2 * b] + outs[2 * b + 1] for b in range(nb)])

    query_mask = np.asarray(inputs["query_mask"])
    if not query_mask.all():
        # masked query rows: reference yields uniform attention over all kv
        for b in range(nb):
            rows = ~query_mask[b]
            if rows.any():
                V = key_value[b] @ w_v  # [tkv, 256]
                meanV = V.mean(axis=0)  # [256]
                group = N_HEADS // NUM_KV_HEADS
                feat = np.concatenate(
                    [
                        meanV.reshape(NUM_KV_HEADS, D_K)[h // group]
                        for h in range(N_HEADS)
                    ]
                )
                full[b, rows, :] = feat @ w_out
    return full.astype(np.float32), res


def kernel(**inputs):
    out, _ = _run(inputs, trace=False)
    return out


def kernel_traced(**inputs):
    out, res = _run(inputs, trace=True)
    return out, res


if __name__ == "__main__":
    print("kernel.py is a library; use test.py")
